# revision 1
# baseline (speedup 1.0000x reference)
"""Causal multi-head self-attention with RoPE on 8 Trainium2 NeuronCores.

Sharding: core c handles batch c//2 and heads 8*(c%2) .. 8*(c%2)+8 (half the
heads of one batch). Host sums the two half-head partial outputs per batch.

Per-core program (1 batch, 8 heads as 4 groups of 2), bf16 matmuls with fp32
PSUM accumulation:
  V:    V[tok, h*64+d] = x @ Wv directly (no PE transposes) -> vsb bf16,
        with a ones column per head (65th) for softmax denominators.
  A(g): Q^T/K^T [128 = 2 heads x (32 even | 32 odd), T] + RoPE
        (cos/sin muls on DVE, cross-term swap via SBUF DMAs).
  B(g): per q-chunk qc (512 q): causal S^T blocks [k=128, q<=512] per head
        (PE row-groups 0-63 / 64-127), causal mask via trib matmul on the
        diagonal staircase, ONE exp per kb on ACT -> pt bf16.
        AV per head: av_h [65, 512] = [values ; denominator] (ones column).
        Normalize: reciprocal (DVE) -> partition_broadcast (Pool) -> mul;
        odd head lands at partitions 0-63 and is DMA-shifted to attn rows
        64-127 (one [64,512] bf16 SBUF DMA per (g, qc)).
  C:    out[tok, :] accumulates attn_g^T @ Wo_g (K=128) over the 4 groups.
"""
import numpy as np
from contextlib import ExitStack

import concourse.bass as bass
import concourse.tile as tile
from concourse import bacc, mybir
from concourse.bass_utils import run_bass_kernel_spmd

F32 = mybir.dt.float32
BF16 = mybir.dt.bfloat16
AF = mybir.ActivationFunctionType

D, H, DK, T, B = 1024, 16, 64, 2048, 4
NCORES = 8
ROPE_THETA = 10000.0
_BUILT = {}


def _build_nc():
    nc = bacc.Bacc("TRN2", target_bir_lowering=False, debug=False,
                   num_devices=NCORES)
    xT = nc.dram_tensor("xT", [D, T], BF16, kind="ExternalInput").ap()
    wq = nc.dram_tensor("wq", [D, 512], BF16, kind="ExternalInput").ap()
    wk = nc.dram_tensor("wk", [D, 512], BF16, kind="ExternalInput").ap()
    wv = nc.dram_tensor("wv", [D, 512], BF16, kind="ExternalInput").ap()
    wo = nc.dram_tensor("wo", [512, D], BF16, kind="ExternalInput").ap()
    cc = nc.dram_tensor("cc", [128, T], BF16, kind="ExternalInput").ap()
    ss = nc.dram_tensor("ss", [128, T], BF16, kind="ExternalInput").ap()
    tri = nc.dram_tensor("tri", [128, 2, 128], BF16, kind="ExternalInput").ap()
    out = nc.dram_tensor("out", [T, D], F32, kind="ExternalOutput").ap()

    x3 = xT.rearrange("(dt p) n -> p dt n", p=128)    # [128, 8, T]
    wq3 = wq.rearrange("(dt p) m -> p dt m", p=128)   # [128, 8, 512]
    wk3 = wk.rearrange("(dt p) m -> p dt m", p=128)
    wv3 = wv.rearrange("(dt p) m -> p dt m", p=128)
    wo3 = wo.rearrange("(g p) m -> p g m", p=128)     # [128, 4, 1024]

    with tile.TileContext(nc) as tc, ExitStack() as ctx:
        consts = ctx.enter_context(tc.tile_pool(name="consts", bufs=1))
        wpool = ctx.enter_context(tc.tile_pool(name="wpool", bufs=1))
        xin = ctx.enter_context(tc.tile_pool(name="xin", bufs=1))
        vtp = ctx.enter_context(tc.tile_pool(name="vtp", bufs=1))
        qkp = ctx.enter_context(tc.tile_pool(name="qkp", bufs=2))
        rope = ctx.enter_context(tc.tile_pool(name="rope", bufs=1))
        atp = ctx.enter_context(tc.tile_pool(name="atp", bufs=1))
        ptp = ctx.enter_context(tc.tile_pool(name="ptp", bufs=3))
        nrm = ctx.enter_context(tc.tile_pool(name="nrm", bufs=2))
        osb = ctx.enter_context(tc.tile_pool(name="osb", bufs=2))
        ps_mix = ctx.enter_context(tc.tile_pool(name="ps_mix", bufs=2, space="PSUM"))
        ps_s = ctx.enter_context(tc.tile_pool(name="ps_s", bufs=2, space="PSUM"))
        ps_av = ctx.enter_context(tc.tile_pool(name="ps_av", bufs=1, space="PSUM"))

        c_cc = consts.tile([128, T], BF16)
        c_ss = consts.tile([128, T], BF16)
        c_tri = consts.tile([128, 2, 128], BF16)
        w_q = wpool.tile([128, 8, 512], BF16)
        w_k = wpool.tile([128, 8, 512], BF16)
        w_v = wpool.tile([128, 8, 512], BF16)
        w_o = wpool.tile([128, 4, D], BF16)
        xt = xin.tile([128, 8, T], BF16)
        vsb = vtp.tile([128, 16, 8, 65], BF16)
        attn = atp.tile([128, 4, T], BF16)

        # V needs w_v + xt first; split per dt-chunk to parallelize queues
        # and let the first V matmuls start as soon as chunk 0 lands.
        # xt lands in per-dt 512-col quarters so V/A consume it as it streams
        for dt_i in range(8):
            nc.sync.dma_start(w_v[:, dt_i, :], wv3[:, dt_i, :])
        for tb in range(4):
            c0 = tb * 512
            for dt_i in range(8):
                nc.sync.dma_start(xt[:, dt_i, c0:c0 + 512],
                                  x3[:, dt_i, c0:c0 + 512])
        nc.sync.dma_start(w_q[:], wq3)
        nc.sync.dma_start(c_cc[:], cc)
        nc.sync.dma_start(c_ss[:], ss)
        nc.sync.dma_start(w_k[:], wk3)
        nc.sync.dma_start(c_tri[:], tri)
        nc.sync.dma_start(w_o[:], wo3)

        # ---------------- V: V[tok, h*64+d] = x @ Wv, written pre-transposed
        nc.gpsimd.memset(vsb[:, :, :, 64:65], 1.0)

        # Warm the PE clock (HAM) during the initial input-DMA wait: dummy
        # matmuls on the memset ones column need no DMA. Sized to end just
        # before the first real matmul's inputs land (~8us) so the HW clock
        # gate is open (and stays open: trailing idle < the 3.4us window).
        ones_col = vsb[:, 0, 0, 64:65]
        ones_row = vsb[:, :, :, 64:65].rearrange("p a b c -> p (a b c)")
        dav = ps_av.tile([128, 512], F32, tag="av0")
        for _ in range(100):
            nc.tensor.matmul(dav[0:1, 0:128], ones_col, ones_row,
                             start=True, stop=True, skip_group_check=True)

        def v_quarter(tb):
            for tt in range(4 * tb, 4 * tb + 4):
                psV = ps_mix.tile([128, 512], F32, tag="mix")
                for dt_i in range(8):
                    nc.tensor.matmul(psV[:],
                                     xt[:, dt_i, tt * 128:(tt + 1) * 128],
                                     w_v[:, dt_i, :],
                                     start=(dt_i == 0), stop=(dt_i == 7))
                src = psV[:].rearrange("p (h d) -> p h d", h=8)
                nc.scalar.activation(vsb[:, tt, :, 0:64], src, AF.Copy)

        def phase_a(g, interleave=None):
            qt = qkp.tile([128, T], BF16, tag="qt")
            kt = qkp.tile([128, T], BF16, tag="kt")
            bq = rope.tile([128, T], BF16, tag="bq")
            bk = rope.tile([128, T], BF16, tag="bk")
            bsq = rope.tile([128, T], BF16, tag="bsq")
            bsk = rope.tile([128, T], BF16, tag="bsk")
            bwq = rope.tile([128, T], BF16, tag="bwq")
            bwk = rope.tile([128, T], BF16, tag="bwk")
            for tb in range(4):
                if interleave is not None:
                    interleave(tb)
                lt = tb * 512
                for wsb, bdst, bsdst, stg in ((w_q, bq, bsq, "sq"),
                                              (w_k, bk, bsk, "sk")):
                    psA = ps_mix.tile([128, 512], F32, tag="mix")
                    for dt_i in range(8):
                        nc.tensor.matmul(
                            psA[:], wsb[:, dt_i, g * 128:(g + 1) * 128],
                            xt[:, dt_i, lt:lt + 512],
                            start=(dt_i == 0), stop=(dt_i == 7))
                    # bf16 staging copy on ACT frees the PSUM slot quickly
                    # and lets both RoPE muls run in the DVE 4x mode.
                    stga = rope.tile([128, 512], BF16, tag=stg)
                    nc.vector.tensor_copy(stga[:], psA[:])
                    nc.vector.tensor_mul(bdst[:, lt:lt + 512], stga[:],
                                         c_cc[:, lt:lt + 512])
                    nc.vector.tensor_mul(bsdst[:, lt:lt + 512], stga[:],
                                         c_ss[:, lt:lt + 512])
                if tb in (1, 3):
                    hlo = (tb - 1) * 512
                    for bt, bw, bb, dest in ((bsq, bwq, bq, qt),
                                             (bsk, bwk, bk, kt)):
                        for hh in range(2):
                            r0 = hh * 64
                            nc.sync.dma_start(
                                bw[r0 + 32:r0 + 64, hlo:hlo + 1024],
                                bt[r0:r0 + 32, hlo:hlo + 1024])
                            nc.sync.dma_start(
                                bw[r0:r0 + 32, hlo:hlo + 1024],
                                bt[r0 + 32:r0 + 64, hlo:hlo + 1024])
                        nc.vector.tensor_add(dest[:, hlo:hlo + 1024],
                                             bb[:, hlo:hlo + 1024],
                                             bw[:, hlo:hlo + 1024])
            return qt, kt

        def phase_b_chunk(g, qt, kt, qc):
            qlo = qc * 512
            av0 = ps_av.tile([128, 512], F32, tag="av0")
            av1 = ps_av.tile([128, 512], F32, tag="av1")
            nkb = 4 * qc + 4

            def emit_av(kb, pt, qq, n, last):
                for h, avt in ((0, av0), (1, av1)):
                    nc.tensor.matmul(avt[0:65, qq:512],
                                     vsb[:, kb, 2 * g + h, :],
                                     pt[:, h, 0:n],
                                     start=(kb == 0), stop=last,
                                     skip_group_check=True)

            # Software-pipelined by one stage: av(kb) is emitted AFTER
            # scores(kb+1) so the in-order PE queue never stalls on exp(kb).
            pend = None
            for kb in range(nkb):
                k0 = kb * 128
                q0 = max(qlo, k0)
                n = qlo + 512 - q0
                qq = q0 - qlo
                diag = (q0 == k0)
                last = (kb == nkb - 1)
                sps = ps_s.tile([128, 2, 512], F32, tag="s")
                for h in range(2):
                    nc.tensor.matmul(
                        sps[:, h, 0:n],
                        kt[h * 64:(h + 1) * 64, k0:k0 + 128],
                        qt[h * 64:(h + 1) * 64, q0:qlo + 512],
                        start=True, stop=True,
                        skip_group_check=True)
                pt = ptp.tile([128, 2, 512], BF16, tag="pt")
                nc.scalar.activation(pt[:, :, 0:n], sps[:, :, 0:n], AF.Exp)
                if diag:
                    # causal mask: zero the upper triangle of the 128-wide
                    # diagonal block (all-bf16 SBUF mul -> DVE 4x mode)
                    nc.vector.tensor_mul(pt[:, :, 0:128], pt[:, :, 0:128],
                                         c_tri[:])
                if pend is not None:
                    emit_av(*pend)
                pend = (kb, pt, qq, n, last)
            emit_av(*pend)
            # Evacuate av banks ASAP (bf16 SBUF copies), then an all-bf16
            # normalization chain: DVE muls hit the 4x mode.
            avc = nrm.tile([128, 2, 512], BF16, tag="avc")
            rec = nrm.tile([128, 2, 512], BF16, tag="rec")
            rec2 = nrm.tile([128, 2, 512], BF16, tag="rec2")
            rbs = nrm.tile([64, 2, 512], BF16, tag="rbs")
            tmp = nrm.tile([64, 512], BF16, tag="tmp")
            nc.scalar.activation(avc[0:65, 0, :], av0[0:65, :], AF.Copy)
            nc.vector.tensor_copy(avc[0:65, 1, :], av1[0:65, :])
            with nc.allow_low_precision(reason="1/denom in bf16: 0.4% uniform row scale, well within tolerance"):
                nc.vector.reciprocal(rec[64:65, :, :], avc[64:65, :, :])
            # partition_broadcast only reads partition 0 on HW — hop there.
            nc.gpsimd.dma_start(rec2[0:1, :, :], rec[64:65, :, :])
            nc.gpsimd.partition_broadcast(rbs[0:64, :, :], rec2[0:1, :, :])
            nc.vector.tensor_mul(attn[0:64, g, qlo:qlo + 512],
                                 avc[0:64, 0, :], rbs[0:64, 0, :])
            nc.vector.tensor_mul(tmp[0:64, :], avc[0:64, 1, :],
                                 rbs[0:64, 1, :])
            nc.sync.dma_start(attn[64:128, g, qlo:qlo + 512], tmp[0:64, :])

        def phase_c(qc):
            for tp2 in range(2 * qc, 2 * qc + 2):
                o_sb = osb.tile([128, 2, D], F32, tag="osb")
                for s in range(2):
                    tt = tp2 * 2 + s
                    for h5 in range(2):
                        psO = ps_mix.tile([128, 512], F32, tag="mix")
                        for g in range(4):
                            nc.tensor.matmul(
                                psO[:], attn[:, g, tt * 128:(tt + 1) * 128],
                                w_o[:, g, h5 * 512:(h5 + 1) * 512],
                                start=(g == 0), stop=(g == 3))
                        dst = o_sb[:, s, h5 * 512:(h5 + 1) * 512]
                        nc.scalar.activation(dst, psO[:], AF.Copy)
                row = tp2 * 256
                nc.sync.dma_start(
                    out[row:row + 256, :].rearrange("(s p) f -> p s f", p=128),
                    o_sb[:])

        # Emission order: A(g+1) interleaves with B(g) so the PE gap-fills;
        # C(qc) right after B(3, qc) — all four groups' attn cols are ready.
        for tb in range(4):
            v_quarter(tb)
        qk = {0: phase_a(0)}
        for g in range(4):
            # g=3 runs qc=0 first (smallest chunk: nothing can fill the PE
            # while it waits on B(3,0)), then descending so C(qc) work is
            # always available to gap-fill the remaining chunks.
            qcs = (0, 3, 2, 1) if g == 3 else range(4)
            for qc in qcs:
                phase_b_chunk(g, *qk[g], qc)
                if g == 3:
                    phase_c(qc)
            if g + 1 <= 3:
                qk[g + 1] = phase_a(g + 1)

    nc.compile()
    return nc


def _host_prep(x, W_qkv, W_o, token_positions):
    import ml_dtypes
    bf = ml_dtypes.bfloat16
    x = np.asarray(x, np.float32)
    W_qkv = np.asarray(W_qkv, np.float32)
    W_o = np.asarray(W_o, np.float32)
    pos = np.asarray(token_positions, np.float64)
    i = np.arange(32)
    inv = 1.0 / (ROPE_THETA ** (2 * i / DK))
    ang = pos[None, :] * inv[:, None]
    CC = np.tile(np.cos(ang), (4, 1)).astype(bf)
    sn = np.sin(ang)
    SS = np.concatenate([sn, -sn, sn, -sn], 0).astype(bf)
    tri01 = np.where(np.arange(128)[:, None] <= np.arange(128)[None, :],
                     1.0, 0.0).astype(bf)
    tri2 = np.repeat(tri01[:, None, :], 2, axis=1)
    xTb = [np.ascontiguousarray(x[b].T).astype(bf) for b in range(B)]
    in_maps = []
    for c in range(NCORES):
        b, hg = c // 2, c % 2
        qcols, vcols = [], []
        for h in range(hg * 8, hg * 8 + 8):
            for half in range(2):
                qcols.extend(h * DK + 2 * ii + half for ii in range(32))
            vcols.extend(h * DK + d for d in range(DK))
        qcols = np.array(qcols)
        vcols = np.array(vcols)
        in_maps.append({
            "xT": xTb[b],
            "wq": np.ascontiguousarray(W_qkv[:, 0 * D + qcols]).astype(bf),
            "wk": np.ascontiguousarray(W_qkv[:, 1 * D + qcols] / 8.0).astype(bf),
            "wv": np.ascontiguousarray(W_qkv[:, 2 * D + vcols]).astype(bf),
            "wo": np.ascontiguousarray(W_o[vcols, :]).astype(bf),
            "cc": CC, "ss": SS, "tri": tri2,
        })
    return in_maps


def kernel(x, W_qkv, W_o, token_positions, _trace=False):
    in_maps = _host_prep(x, W_qkv, W_o, token_positions)
    if "nc" not in _BUILT:
        _BUILT["nc"] = _build_nc()
    res = run_bass_kernel_spmd(_BUILT["nc"], in_maps,
                               core_ids=list(range(NCORES)), trace=_trace)
    _BUILT["last_result"] = res
    total = np.zeros((B, T, D), np.float32)
    for c in range(NCORES):
        total[c // 2] += res.results[c]["out"]
    return total



# revision 7
# speedup vs baseline: 1.2739x; 1.2739x over previous
"""Causal multi-head self-attention with RoPE on 8 Trainium2 NeuronCores.

Sharding: core c handles batch c//2 and heads 8*(c%2) .. 8*(c%2)+8 (half the
heads of one batch). Host sums the two half-head partial outputs per batch.

Per-core program (1 batch, 8 heads as 4 groups of 2), bf16 matmuls with fp32
PSUM accumulation:
  V:    V[tok, h*64+d] = x @ Wv directly (no PE transposes) -> vsb bf16,
        with a ones column per head (65th) for softmax denominators.
  A(g): Q^T/K^T [128 = 2 heads x (32 even | 32 odd), T] + RoPE
        (cos/sin muls on DVE, cross-term swap via SBUF DMAs).
  B(g): per q-chunk qc (512 q): causal S^T blocks [k=128, q<=512] per head
        (PE row-groups 0-63 / 64-127), causal mask via trib matmul on the
        diagonal staircase, ONE exp per kb on ACT -> pt bf16.
        AV per head: av_h [65, 512] = [values ; denominator] (ones column).
        Normalize: reciprocal (DVE) -> partition_broadcast (Pool) -> mul;
        odd head lands at partitions 0-63 and is DMA-shifted to attn rows
        64-127 (one [64,512] bf16 SBUF DMA per (g, qc)).
  C:    out[tok, :] accumulates attn_g^T @ Wo_g (K=128) over the 4 groups.
"""
import numpy as np
from contextlib import ExitStack

import concourse.bass as bass
import concourse.tile as tile
from concourse import bacc, mybir
from concourse.bass_utils import run_bass_kernel_spmd

F32 = mybir.dt.float32
BF16 = mybir.dt.bfloat16
AF = mybir.ActivationFunctionType

D, H, DK, T, B = 1024, 16, 64, 2048, 4
NCORES = 8
ROPE_THETA = 10000.0
_BUILT = {}


def _build_nc():
    nc = bacc.Bacc("TRN2", target_bir_lowering=False, debug=False,
                   num_devices=NCORES)
    xT = nc.dram_tensor("xT", [D, T], BF16, kind="ExternalInput").ap()
    wq = nc.dram_tensor("wq", [D, 512], BF16, kind="ExternalInput").ap()
    wk = nc.dram_tensor("wk", [D, 512], BF16, kind="ExternalInput").ap()
    wv = nc.dram_tensor("wv", [D, 512], BF16, kind="ExternalInput").ap()
    wo = nc.dram_tensor("wo", [512, D], BF16, kind="ExternalInput").ap()
    cc = nc.dram_tensor("cc", [128, T], BF16, kind="ExternalInput").ap()
    ss = nc.dram_tensor("ss", [128, T], BF16, kind="ExternalInput").ap()
    tri = nc.dram_tensor("tri", [128, 2, 128], BF16, kind="ExternalInput").ap()
    out = nc.dram_tensor("out", [T, D], F32, kind="ExternalOutput").ap()

    x3 = xT.rearrange("(dt p) n -> p dt n", p=128)    # [128, 8, T]
    wq3 = wq.rearrange("(dt p) m -> p dt m", p=128)   # [128, 8, 512]
    wk3 = wk.rearrange("(dt p) m -> p dt m", p=128)
    wv3 = wv.rearrange("(dt p) m -> p dt m", p=128)
    wo3 = wo.rearrange("(g p) m -> p g m", p=128)     # [128, 4, 1024]

    with tile.TileContext(nc) as tc, ExitStack() as ctx:
        consts = ctx.enter_context(tc.tile_pool(name="consts", bufs=1))
        wpool = ctx.enter_context(tc.tile_pool(name="wpool", bufs=1))
        xin = ctx.enter_context(tc.tile_pool(name="xin", bufs=1))
        vtp = ctx.enter_context(tc.tile_pool(name="vtp", bufs=1))
        qkp = ctx.enter_context(tc.tile_pool(name="qkp", bufs=2))
        rope = ctx.enter_context(tc.tile_pool(name="rope", bufs=1))
        atp = ctx.enter_context(tc.tile_pool(name="atp", bufs=1))
        ptp = ctx.enter_context(tc.tile_pool(name="ptp", bufs=3))
        nrm = ctx.enter_context(tc.tile_pool(name="nrm", bufs=2))
        osb = ctx.enter_context(tc.tile_pool(name="osb", bufs=2))
        ps_mix = ctx.enter_context(tc.tile_pool(name="ps_mix", bufs=2, space="PSUM"))
        ps_s = ctx.enter_context(tc.tile_pool(name="ps_s", bufs=2, space="PSUM"))
        ps_av = ctx.enter_context(tc.tile_pool(name="ps_av", bufs=1, space="PSUM"))

        c_cc = consts.tile([128, T], BF16)
        c_ss = consts.tile([128, T], BF16)
        c_tri = consts.tile([128, 2, 128], BF16)
        w_q = wpool.tile([128, 8, 512], BF16)
        w_k = wpool.tile([128, 8, 512], BF16)
        w_v = wpool.tile([128, 8, 512], BF16)
        w_o = wpool.tile([128, 4, D], BF16)
        xt = xin.tile([128, 8, T], BF16)
        vsb = vtp.tile([128, 16, 8, 65], BF16)
        attn = atp.tile([128, 4, T], BF16)

        # V needs w_v + xt first; split per dt-chunk to parallelize queues
        # and let the first V matmuls start as soon as chunk 0 lands.
        # xt lands in per-dt 512-col quarters so V/A consume it as it streams
        for dt_i in range(8):
            nc.sync.dma_start(w_v[:, dt_i, :], wv3[:, dt_i, :])
        for tb in range(4):
            c0 = tb * 512
            for dt_i in range(8):
                nc.sync.dma_start(xt[:, dt_i, c0:c0 + 512],
                                  x3[:, dt_i, c0:c0 + 512])
        nc.sync.dma_start(w_q[:], wq3)
        nc.sync.dma_start(c_cc[:], cc)
        nc.sync.dma_start(c_ss[:], ss)
        nc.sync.dma_start(w_k[:], wk3)
        nc.sync.dma_start(c_tri[:], tri)
        nc.sync.dma_start(w_o[:], wo3)

        # ---------------- V: V[tok, h*64+d] = x @ Wv, written pre-transposed
        nc.gpsimd.memset(vsb[:, :, :, 64:65], 1.0)

        # Warm the PE clock (HAM) during the initial input-DMA wait: dummy
        # matmuls on the memset ones column need no DMA. Sized to end just
        # before the first real matmul's inputs land (~10us at ~270ns each)
        # so the HW clock gate is open (and stays open: trailing idle < the
        # 3.4us window).
        ones_col = vsb[:, 0, 0, 64:65]
        ones_row = vsb[:, :, :, 64:65].rearrange("p a b c -> p (a b c)")
        dav = ps_av.tile([128, 2, 512], F32, tag="av")
        for _ in range(40):
            nc.tensor.matmul(dav[0:1, 0, 0:128], ones_col, ones_row,
                             start=True, stop=True, skip_group_check=True)

        def v_quarter(tb):
            for tt in range(4 * tb, 4 * tb + 4):
                psV = ps_mix.tile([128, 512], F32, tag="mix")
                for dt_i in range(8):
                    nc.tensor.matmul(psV[:],
                                     xt[:, dt_i, tt * 128:(tt + 1) * 128],
                                     w_v[:, dt_i, :],
                                     start=(dt_i == 0), stop=(dt_i == 7))
                src = psV[:].rearrange("p (h d) -> p h d", h=8)
                nc.scalar.activation(vsb[:, tt, :, 0:64], src, AF.Copy)

        def phase_a(g, interleave=None):
            qt = qkp.tile([128, T], BF16, tag="qt")
            kt = qkp.tile([128, T], BF16, tag="kt")
            bq = rope.tile([128, T], BF16, tag="bq")
            bk = rope.tile([128, T], BF16, tag="bk")
            bsq = rope.tile([128, T], BF16, tag="bsq")
            bsk = rope.tile([128, T], BF16, tag="bsk")
            bwq = rope.tile([128, T], BF16, tag="bwq")
            bwk = rope.tile([128, T], BF16, tag="bwk")
            for tb in range(4):
                if interleave is not None:
                    interleave(tb)
                lt = tb * 512
                for wsb, bdst, bsdst, stg in ((w_q, bq, bsq, "sq"),
                                              (w_k, bk, bsk, "sk")):
                    psA = ps_mix.tile([128, 512], F32, tag="mix")
                    for dt_i in range(8):
                        nc.tensor.matmul(
                            psA[:], wsb[:, dt_i, g * 128:(g + 1) * 128],
                            xt[:, dt_i, lt:lt + 512],
                            start=(dt_i == 0), stop=(dt_i == 7))
                    # bf16 staging copy on ACT frees the PSUM slot quickly
                    # and lets both RoPE muls run in the DVE 4x mode.
                    stga = rope.tile([128, 512], BF16, tag=stg)
                    nc.vector.tensor_copy(stga[:], psA[:])
                    nc.vector.tensor_mul(bdst[:, lt:lt + 512], stga[:],
                                         c_cc[:, lt:lt + 512])
                    nc.vector.tensor_mul(bsdst[:, lt:lt + 512], stga[:],
                                         c_ss[:, lt:lt + 512])
                if tb in (1, 3):
                    hlo = (tb - 1) * 512
                    for bt, bw, bb, dest in ((bsq, bwq, bq, qt),
                                             (bsk, bwk, bk, kt)):
                        for hh in range(2):
                            r0 = hh * 64
                            nc.sync.dma_start(
                                bw[r0 + 32:r0 + 64, hlo:hlo + 1024],
                                bt[r0:r0 + 32, hlo:hlo + 1024])
                            nc.sync.dma_start(
                                bw[r0:r0 + 32, hlo:hlo + 1024],
                                bt[r0 + 32:r0 + 64, hlo:hlo + 1024])
                        nc.vector.tensor_add(dest[:, hlo:hlo + 1024],
                                             bb[:, hlo:hlo + 1024],
                                             bw[:, hlo:hlo + 1024])
            return qt, kt

        def phase_b_chunk(g, qt, kt, qc):
            qlo = qc * 512
            av = ps_av.tile([128, 2, 512], F32, tag="av")
            nkb = 4 * qc + 4

            def emit_av(kb, pt, qq, n, last):
                for h in range(2):
                    nc.tensor.matmul(av[0:65, h, qq:512],
                                     vsb[:, kb, 2 * g + h, :],
                                     pt[:, h, 0:n],
                                     start=(kb == 0), stop=last,
                                     skip_group_check=True)

            # Software-pipelined by one stage: av(kb) is emitted AFTER
            # scores(kb+1) so the in-order PE queue never stalls on exp(kb).
            pend = None
            for kb in range(nkb):
                k0 = kb * 128
                q0 = max(qlo, k0)
                n = qlo + 512 - q0
                qq = q0 - qlo
                diag = (q0 == k0)
                last = (kb == nkb - 1)
                sps = ps_s.tile([128, 2, 512], F32, tag="s")
                for h in range(2):
                    nc.tensor.matmul(
                        sps[:, h, 0:n],
                        kt[h * 64:(h + 1) * 64, k0:k0 + 128],
                        qt[h * 64:(h + 1) * 64, q0:qlo + 512],
                        start=True, stop=True,
                        skip_group_check=True)
                pt = ptp.tile([128, 2, 512], BF16, tag="pt")
                nc.scalar.activation(pt[:, :, 0:n], sps[:, :, 0:n], AF.Exp)
                if diag:
                    # causal mask: zero the upper triangle of the 128-wide
                    # diagonal block (all-bf16 SBUF mul -> DVE 4x mode)
                    nc.vector.tensor_mul(pt[:, :, 0:128], pt[:, :, 0:128],
                                         c_tri[:])
                if pend is not None:
                    emit_av(*pend)
                pend = (kb, pt, qq, n, last)
            emit_av(*pend)
            # Evacuate av banks ASAP (bf16 SBUF copies). Normalization:
            # fast approx reciprocal of the denominator row (f32, direct
            # from PSUM), then one SWDGE DMA that broadcasts it across 64
            # partitions AND casts f32->bf16, then bf16 DVE muls (2x mode).
            avc = nrm.tile([128, 2, 512], BF16, tag="avc")
            dens = nrm.tile([1, 2, 512], F32, tag="dens")
            rec = nrm.tile([1, 2, 512], F32, tag="rec")
            recb = nrm.tile([1, 2, 512], BF16, tag="recb")
            rbs = nrm.tile([64, 2, 512], BF16, tag="rbs")
            tmp = nrm.tile([64, 512], BF16, tag="tmp")
            nc.scalar.activation(avc[0:65, 0, :], av[0:65, 0, :], AF.Copy)
            nc.vector.tensor_copy(avc[0:65, 1, :], av[0:65, 1, :])
            # custom-DVE ops read SBUF; stage the PSUM denominator row first
            nc.vector.tensor_copy(dens[0:1, :, :], av[64:65, :, :])
            nc.vector.reciprocal_approx_fast(rec[0:1, :, :], dens[0:1, :, :])
            with nc.allow_low_precision(reason="1/denom in bf16: 0.4% uniform row scale, well within tolerance"):
                nc.vector.tensor_copy(recb[0:1, :, :], rec[0:1, :, :])
            nc.gpsimd.partition_broadcast(rbs[0:64, :, :], recb[0:1, :, :])
            nc.vector.tensor_mul(attn[0:64, g, qlo:qlo + 512],
                                 avc[0:64, 0, :], rbs[0:64, 0, :])
            nc.vector.tensor_mul(tmp[0:64, :], avc[0:64, 1, :],
                                 rbs[0:64, 1, :])
            nc.sync.dma_start(attn[64:128, g, qlo:qlo + 512], tmp[0:64, :])

        def phase_c(qc):
            for tp2 in range(2 * qc, 2 * qc + 2):
                o_sb = osb.tile([128, 2, D], F32, tag="osb")
                for s in range(2):
                    tt = tp2 * 2 + s
                    for h5 in range(2):
                        psO = ps_mix.tile([128, 512], F32, tag="mix")
                        for g in range(4):
                            nc.tensor.matmul(
                                psO[:], attn[:, g, tt * 128:(tt + 1) * 128],
                                w_o[:, g, h5 * 512:(h5 + 1) * 512],
                                start=(g == 0), stop=(g == 3))
                        dst = o_sb[:, s, h5 * 512:(h5 + 1) * 512]
                        # split PSUM evacuation between ACT and DVE so
                        # neither engine becomes the bottleneck
                        if s == 0:
                            nc.scalar.activation(dst, psO[:], AF.Copy)
                        else:
                            nc.vector.tensor_copy(dst, psO[:])
                row = tp2 * 256
                nc.sync.dma_start(
                    out[row:row + 256, :].rearrange("(s p) f -> p s f", p=128),
                    o_sb[:])

        # Emission order: A(g+1) interleaves with B(g) so the PE gap-fills;
        # C(qc) right after B(3, qc) — all four groups' attn cols are ready.
        for tb in range(4):
            v_quarter(tb)
        qk = {0: phase_a(0)}
        for g in range(4):
            # g=3 runs qc=0 first (smallest chunk: nothing can fill the PE
            # while it waits on B(3,0)), then descending so C(qc) work is
            # always available to gap-fill the remaining chunks.
            qcs = (0, 3, 2, 1) if g == 3 else range(4)
            for qc in qcs:
                phase_b_chunk(g, *qk[g], qc)
                if g == 3:
                    phase_c(qc)
            if g + 1 <= 3:
                qk[g + 1] = phase_a(g + 1)

    nc.compile()
    return nc


def _host_prep(x, W_qkv, W_o, token_positions):
    import ml_dtypes
    bf = ml_dtypes.bfloat16
    x = np.asarray(x, np.float32)
    W_qkv = np.asarray(W_qkv, np.float32)
    W_o = np.asarray(W_o, np.float32)
    pos = np.asarray(token_positions, np.float64)
    i = np.arange(32)
    inv = 1.0 / (ROPE_THETA ** (2 * i / DK))
    ang = pos[None, :] * inv[:, None]
    CC = np.tile(np.cos(ang), (4, 1)).astype(bf)
    sn = np.sin(ang)
    SS = np.concatenate([sn, -sn, sn, -sn], 0).astype(bf)
    tri01 = np.where(np.arange(128)[:, None] <= np.arange(128)[None, :],
                     1.0, 0.0).astype(bf)
    tri2 = np.repeat(tri01[:, None, :], 2, axis=1)
    xTb = [np.ascontiguousarray(x[b].T).astype(bf) for b in range(B)]
    in_maps = []
    for c in range(NCORES):
        b, hg = c // 2, c % 2
        qcols, vcols = [], []
        for h in range(hg * 8, hg * 8 + 8):
            for half in range(2):
                qcols.extend(h * DK + 2 * ii + half for ii in range(32))
            vcols.extend(h * DK + d for d in range(DK))
        qcols = np.array(qcols)
        vcols = np.array(vcols)
        in_maps.append({
            "xT": xTb[b],
            "wq": np.ascontiguousarray(W_qkv[:, 0 * D + qcols]).astype(bf),
            "wk": np.ascontiguousarray(W_qkv[:, 1 * D + qcols] / 8.0).astype(bf),
            "wv": np.ascontiguousarray(W_qkv[:, 2 * D + vcols]).astype(bf),
            "wo": np.ascontiguousarray(W_o[vcols, :]).astype(bf),
            "cc": CC, "ss": SS, "tri": tri2,
        })
    return in_maps


def kernel(x, W_qkv, W_o, token_positions, _trace=False):
    in_maps = _host_prep(x, W_qkv, W_o, token_positions)
    if "nc" not in _BUILT:
        _BUILT["nc"] = _build_nc()
    res = run_bass_kernel_spmd(_BUILT["nc"], in_maps,
                               core_ids=list(range(NCORES)), trace=_trace)
    _BUILT["last_result"] = res
    total = np.zeros((B, T, D), np.float32)
    for c in range(NCORES):
        total[c // 2] += res.results[c]["out"]
    return total



# revision 12
# speedup vs baseline: 1.2829x; 1.0070x over previous
"""Causal multi-head self-attention with RoPE on 8 Trainium2 NeuronCores.

Sharding: core c handles batch c//2 and heads 8*(c%2) .. 8*(c%2)+8 (half the
heads of one batch). Host sums the two half-head partial outputs per batch.

Per-core program (1 batch, 8 heads as 4 groups of 2), bf16 matmuls with fp32
PSUM accumulation:
  V:    V[tok, h*64+d] = x @ Wv directly (no PE transposes) -> vsb bf16,
        with a ones column per head (65th) for softmax denominators.
  A(g): Q^T/K^T [128 = 2 heads x (32 even | 32 odd), T] + RoPE
        (cos/sin muls on DVE, cross-term swap via SBUF DMAs).
  B(g): per q-chunk qc (512 q): causal S^T blocks [k=128, q<=512] per head
        (PE row-groups 0-63 / 64-127), causal mask via trib matmul on the
        diagonal staircase, ONE exp per kb on ACT -> pt bf16.
        AV per head: av_h [65, 512] = [values ; denominator] (ones column).
        Normalize: reciprocal (DVE) -> partition_broadcast (Pool) -> mul;
        odd head lands at partitions 0-63 and is DMA-shifted to attn rows
        64-127 (one [64,512] bf16 SBUF DMA per (g, qc)).
  C:    out[tok, :] accumulates attn_g^T @ Wo_g (K=128) over the 4 groups.
"""
import numpy as np
from contextlib import ExitStack

import concourse.bass as bass
import concourse.tile as tile
from concourse import bacc, mybir
from concourse.bass_utils import run_bass_kernel_spmd

F32 = mybir.dt.float32
BF16 = mybir.dt.bfloat16
AF = mybir.ActivationFunctionType

D, H, DK, T, B = 1024, 16, 64, 2048, 4
NCORES = 8
ROPE_THETA = 10000.0
_BUILT = {}


def _build_nc():
    nc = bacc.Bacc("TRN2", target_bir_lowering=False, debug=False,
                   num_devices=NCORES)
    xT = nc.dram_tensor("xT", [D, T], BF16, kind="ExternalInput").ap()
    wq = nc.dram_tensor("wq", [D, 512], BF16, kind="ExternalInput").ap()
    wk = nc.dram_tensor("wk", [D, 512], BF16, kind="ExternalInput").ap()
    wv = nc.dram_tensor("wv", [D, 512], BF16, kind="ExternalInput").ap()
    wo = nc.dram_tensor("wo", [512, D], BF16, kind="ExternalInput").ap()
    cc = nc.dram_tensor("cc", [128, T], BF16, kind="ExternalInput").ap()
    ss = nc.dram_tensor("ss", [128, T], BF16, kind="ExternalInput").ap()
    tri = nc.dram_tensor("tri", [128, 2, 128], BF16, kind="ExternalInput").ap()
    out = nc.dram_tensor("out", [T, D], BF16, kind="ExternalOutput").ap()

    x3 = xT.rearrange("(dt p) n -> p dt n", p=128)    # [128, 8, T]
    wq3 = wq.rearrange("(dt p) m -> p dt m", p=128)   # [128, 8, 512]
    wk3 = wk.rearrange("(dt p) m -> p dt m", p=128)
    wv3 = wv.rearrange("(dt p) m -> p dt m", p=128)
    wo3 = wo.rearrange("(g p) m -> p g m", p=128)     # [128, 4, 1024]

    with tile.TileContext(nc) as tc, ExitStack() as ctx:
        consts = ctx.enter_context(tc.tile_pool(name="consts", bufs=1))
        wpool = ctx.enter_context(tc.tile_pool(name="wpool", bufs=1))
        xin = ctx.enter_context(tc.tile_pool(name="xin", bufs=1))
        vtp = ctx.enter_context(tc.tile_pool(name="vtp", bufs=1))
        qkp = ctx.enter_context(tc.tile_pool(name="qkp", bufs=2))
        rope = ctx.enter_context(tc.tile_pool(name="rope", bufs=1))
        atp = ctx.enter_context(tc.tile_pool(name="atp", bufs=1))
        ptp = ctx.enter_context(tc.tile_pool(name="ptp", bufs=3))
        nrm = ctx.enter_context(tc.tile_pool(name="nrm", bufs=2))
        osb = ctx.enter_context(tc.tile_pool(name="osb", bufs=2))
        ps_mix = ctx.enter_context(tc.tile_pool(name="ps_mix", bufs=2, space="PSUM"))
        ps_s = ctx.enter_context(tc.tile_pool(name="ps_s", bufs=2, space="PSUM"))
        ps_av = ctx.enter_context(tc.tile_pool(name="ps_av", bufs=1, space="PSUM"))

        c_cc = consts.tile([128, T], BF16)
        c_ss = consts.tile([128, T], BF16)
        c_tri = consts.tile([128, 2, 128], BF16)
        w_q = wpool.tile([128, 8, 512], BF16)
        w_k = wpool.tile([128, 8, 512], BF16)
        w_v = wpool.tile([128, 8, 512], BF16)
        w_o = wpool.tile([128, 4, D], BF16)
        xt = xin.tile([128, 8, T], BF16)
        vsb = vtp.tile([128, 16, 8, 65], BF16)
        attn = atp.tile([128, 4, T], BF16)

        # V needs w_v + xt first; split per dt-chunk to parallelize queues
        # and let the first V matmuls start as soon as chunk 0 lands.
        # xt lands in per-dt 512-col quarters so V/A consume it as it streams
        for dt_i in range(8):
            nc.sync.dma_start(w_v[:, dt_i, :], wv3[:, dt_i, :])
        for tb in range(4):
            c0 = tb * 512
            for dt_i in range(8):
                nc.sync.dma_start(xt[:, dt_i, c0:c0 + 512],
                                  x3[:, dt_i, c0:c0 + 512])
        nc.sync.dma_start(w_q[:], wq3)
        nc.sync.dma_start(c_cc[:], cc)
        nc.sync.dma_start(c_ss[:], ss)
        nc.sync.dma_start(w_k[:], wk3)
        nc.sync.dma_start(c_tri[:], tri)
        nc.sync.dma_start(w_o[:], wo3)

        # ---------------- V: V[tok, h*64+d] = x @ Wv, written pre-transposed
        nc.gpsimd.memset(vsb[:, :, :, 64:65], 1.0)

        # Warm the PE clock (HAM) during the initial input-DMA wait: dummy
        # matmuls on the memset ones column need no DMA. Sized to end just
        # before the first real matmul's inputs land (~10us at ~270ns each)
        # so the HW clock gate is open (and stays open: trailing idle < the
        # 3.4us window).
        ones_col = vsb[:, 0, 0, 64:65]
        ones_row = vsb[:, :, :, 64:65].rearrange("p a b c -> p (a b c)")
        dav = ps_av.tile([128, 2, 512], F32, tag="av")
        for _ in range(30):
            nc.tensor.matmul(dav[0:1, 0, 0:128], ones_col, ones_row,
                             start=True, stop=True, skip_group_check=True)

        def v_quarter(tb):
            for tt in range(4 * tb, 4 * tb + 4):
                psV = ps_mix.tile([128, 512], F32, tag="mix")
                for dt_i in range(8):
                    nc.tensor.matmul(psV[:],
                                     xt[:, dt_i, tt * 128:(tt + 1) * 128],
                                     w_v[:, dt_i, :],
                                     start=(dt_i == 0), stop=(dt_i == 7))
                src = psV[:].rearrange("p (h d) -> p h d", h=8)
                nc.scalar.activation(vsb[:, tt, :, 0:64], src, AF.Copy)

        def phase_a(g, interleave=None):
            qt = qkp.tile([128, T], BF16, tag="qt")
            kt = qkp.tile([128, T], BF16, tag="kt")
            bq = rope.tile([128, T], BF16, tag="bq")
            bk = rope.tile([128, T], BF16, tag="bk")
            bsq = rope.tile([128, T], BF16, tag="bsq")
            bsk = rope.tile([128, T], BF16, tag="bsk")
            bwq = rope.tile([128, T], BF16, tag="bwq")
            bwk = rope.tile([128, T], BF16, tag="bwk")
            for tb in range(4):
                if interleave is not None:
                    interleave(tb)
                lt = tb * 512
                for wsb, bdst, bsdst, stg in ((w_q, bq, bsq, "sq"),
                                              (w_k, bk, bsk, "sk")):
                    psA = ps_mix.tile([128, 512], F32, tag="mix")
                    for dt_i in range(8):
                        nc.tensor.matmul(
                            psA[:], wsb[:, dt_i, g * 128:(g + 1) * 128],
                            xt[:, dt_i, lt:lt + 512],
                            start=(dt_i == 0), stop=(dt_i == 7))
                    # bf16 staging copy on ACT frees the PSUM slot quickly
                    # and lets both RoPE muls run in the DVE 4x mode.
                    stga = rope.tile([128, 512], BF16, tag=stg)
                    nc.vector.tensor_copy(stga[:], psA[:])
                    nc.vector.tensor_mul(bdst[:, lt:lt + 512], stga[:],
                                         c_cc[:, lt:lt + 512])
                    nc.vector.tensor_mul(bsdst[:, lt:lt + 512], stga[:],
                                         c_ss[:, lt:lt + 512])
                if tb in (1, 3):
                    hlo = (tb - 1) * 512
                    for bt, bw, bb, dest in ((bsq, bwq, bq, qt),
                                             (bsk, bwk, bk, kt)):
                        for hh in range(2):
                            r0 = hh * 64
                            nc.sync.dma_start(
                                bw[r0 + 32:r0 + 64, hlo:hlo + 1024],
                                bt[r0:r0 + 32, hlo:hlo + 1024])
                            nc.sync.dma_start(
                                bw[r0:r0 + 32, hlo:hlo + 1024],
                                bt[r0 + 32:r0 + 64, hlo:hlo + 1024])
                        nc.vector.tensor_add(dest[:, hlo:hlo + 1024],
                                             bb[:, hlo:hlo + 1024],
                                             bw[:, hlo:hlo + 1024])
            return qt, kt

        def phase_b_chunk(g, qt, kt, qc):
            qlo = qc * 512
            av = ps_av.tile([128, 2, 512], F32, tag="av")
            nkb = 4 * qc + 4

            def emit_av(kb, pt, qq, n, last):
                for h in range(2):
                    nc.tensor.matmul(av[0:65, h, qq:512],
                                     vsb[:, kb, 2 * g + h, :],
                                     pt[:, h, 0:n],
                                     start=(kb == 0), stop=last,
                                     skip_group_check=True)

            # Software-pipelined by one stage: av(kb) is emitted AFTER
            # scores(kb+1) so the in-order PE queue never stalls on exp(kb).
            pend = None
            for kb in range(nkb):
                k0 = kb * 128
                q0 = max(qlo, k0)
                n = qlo + 512 - q0
                qq = q0 - qlo
                diag = (q0 == k0)
                last = (kb == nkb - 1)
                sps = ps_s.tile([128, 2, 512], F32, tag="s")
                for h in range(2):
                    nc.tensor.matmul(
                        sps[:, h, 0:n],
                        kt[h * 64:(h + 1) * 64, k0:k0 + 128],
                        qt[h * 64:(h + 1) * 64, q0:qlo + 512],
                        start=True, stop=True,
                        skip_group_check=True)
                pt = ptp.tile([128, 2, 512], BF16, tag="pt")
                nc.scalar.activation(pt[:, :, 0:n], sps[:, :, 0:n], AF.Exp)
                if diag:
                    # causal mask: zero the upper triangle of the 128-wide
                    # diagonal block (all-bf16 SBUF mul -> DVE 4x mode)
                    nc.vector.tensor_mul(pt[:, :, 0:128], pt[:, :, 0:128],
                                         c_tri[:])
                if pend is not None:
                    emit_av(*pend)
                pend = (kb, pt, qq, n, last)
            emit_av(*pend)
            # Evacuate av banks ASAP (bf16 SBUF copies). Normalization:
            # fast approx reciprocal of the denominator row (f32, direct
            # from PSUM), then one SWDGE DMA that broadcasts it across 64
            # partitions AND casts f32->bf16, then bf16 DVE muls (2x mode).
            avc = nrm.tile([128, 2, 512], BF16, tag="avc")
            dens = nrm.tile([1, 2, 512], F32, tag="dens")
            rec = nrm.tile([1, 2, 512], F32, tag="rec")
            recb = nrm.tile([1, 2, 512], BF16, tag="recb")
            rbs = nrm.tile([64, 2, 512], BF16, tag="rbs")
            tmp = nrm.tile([64, 512], BF16, tag="tmp")
            nc.scalar.activation(avc[0:65, 0, :], av[0:65, 0, :], AF.Copy)
            nc.vector.tensor_copy(avc[0:65, 1, :], av[0:65, 1, :])
            # custom-DVE ops read SBUF; stage the PSUM denominator row first
            nc.vector.tensor_copy(dens[0:1, :, :], av[64:65, :, :])
            nc.vector.reciprocal_approx_fast(rec[0:1, :, :], dens[0:1, :, :])
            with nc.allow_low_precision(reason="1/denom in bf16: 0.4% uniform row scale, well within tolerance"):
                nc.vector.tensor_copy(recb[0:1, :, :], rec[0:1, :, :])
            nc.gpsimd.partition_broadcast(rbs[0:64, :, :], recb[0:1, :, :])
            nc.vector.tensor_mul(attn[0:64, g, qlo:qlo + 512],
                                 avc[0:64, 0, :], rbs[0:64, 0, :])
            nc.vector.tensor_mul(tmp[0:64, :], avc[0:64, 1, :],
                                 rbs[0:64, 1, :])
            nc.sync.dma_start(attn[64:128, g, qlo:qlo + 512], tmp[0:64, :])

        def phase_c(qc):
            for tp2 in range(2 * qc, 2 * qc + 2):
                o_sb = osb.tile([128, 2, D], BF16, tag="osb")
                for s in range(2):
                    tt = tp2 * 2 + s
                    for h5 in range(2):
                        psO = ps_mix.tile([128, 512], F32, tag="mix")
                        for g in range(4):
                            nc.tensor.matmul(
                                psO[:], attn[:, g, tt * 128:(tt + 1) * 128],
                                w_o[:, g, h5 * 512:(h5 + 1) * 512],
                                start=(g == 0), stop=(g == 3))
                        dst = o_sb[:, s, h5 * 512:(h5 + 1) * 512]
                        # split PSUM evacuation between ACT and DVE so
                        # neither engine becomes the bottleneck
                        if s == 0:
                            nc.scalar.activation(dst, psO[:], AF.Copy)
                        else:
                            nc.vector.tensor_copy(dst, psO[:])
                row = tp2 * 256
                nc.sync.dma_start(
                    out[row:row + 256, :].rearrange("(s p) f -> p s f", p=128),
                    o_sb[:])

        # Emission order: A(g+1) interleaves with B(g) so the PE gap-fills;
        # C(qc) right after B(3, qc) — all four groups' attn cols are ready.
        for tb in range(4):
            v_quarter(tb)
        qk = {0: phase_a(0)}
        for g in range(4):
            # g=3 runs qc=0 first (smallest chunk: nothing can fill the PE
            # while it waits on B(3,0)), then descending so C(qc) work is
            # always available to gap-fill the remaining chunks. C(0) is
            # held back to the very end: its deps are ready long before,
            # so it fills the PE while the last chunk's normalize tail
            # (recip -> broadcast -> mul) finishes.
            qcs = (0, 3, 2, 1) if g == 3 else range(4)
            for qc in qcs:
                phase_b_chunk(g, *qk[g], qc)
                if g == 3 and qc != 0:
                    phase_c(qc)
            if g + 1 <= 3:
                qk[g + 1] = phase_a(g + 1)
        phase_c(0)

    nc.compile()
    return nc


def _host_prep(x, W_qkv, W_o, token_positions):
    import ml_dtypes
    bf = ml_dtypes.bfloat16
    x = np.asarray(x, np.float32)
    W_qkv = np.asarray(W_qkv, np.float32)
    W_o = np.asarray(W_o, np.float32)
    pos = np.asarray(token_positions, np.float64)
    i = np.arange(32)
    inv = 1.0 / (ROPE_THETA ** (2 * i / DK))
    ang = pos[None, :] * inv[:, None]
    CC = np.tile(np.cos(ang), (4, 1)).astype(bf)
    sn = np.sin(ang)
    SS = np.concatenate([sn, -sn, sn, -sn], 0).astype(bf)
    tri01 = np.where(np.arange(128)[:, None] <= np.arange(128)[None, :],
                     1.0, 0.0).astype(bf)
    tri2 = np.repeat(tri01[:, None, :], 2, axis=1)
    xTb = [np.ascontiguousarray(x[b].T).astype(bf) for b in range(B)]
    in_maps = []
    for c in range(NCORES):
        b, hg = c // 2, c % 2
        qcols, vcols = [], []
        for h in range(hg * 8, hg * 8 + 8):
            for half in range(2):
                qcols.extend(h * DK + 2 * ii + half for ii in range(32))
            vcols.extend(h * DK + d for d in range(DK))
        qcols = np.array(qcols)
        vcols = np.array(vcols)
        in_maps.append({
            "xT": xTb[b],
            "wq": np.ascontiguousarray(W_qkv[:, 0 * D + qcols]).astype(bf),
            "wk": np.ascontiguousarray(W_qkv[:, 1 * D + qcols] / 8.0).astype(bf),
            "wv": np.ascontiguousarray(W_qkv[:, 2 * D + vcols]).astype(bf),
            "wo": np.ascontiguousarray(W_o[vcols, :]).astype(bf),
            "cc": CC, "ss": SS, "tri": tri2,
        })
    return in_maps


def kernel(x, W_qkv, W_o, token_positions, _trace=False):
    in_maps = _host_prep(x, W_qkv, W_o, token_positions)
    if "nc" not in _BUILT:
        _BUILT["nc"] = _build_nc()
    res = run_bass_kernel_spmd(_BUILT["nc"], in_maps,
                               core_ids=list(range(NCORES)), trace=_trace)
    _BUILT["last_result"] = res
    total = np.zeros((B, T, D), np.float32)
    for c in range(NCORES):
        total[c // 2] += np.asarray(res.results[c]["out"], np.float32)
    return total



# revision 14
# speedup vs baseline: 1.2989x; 1.0125x over previous
"""Causal multi-head self-attention with RoPE on 8 Trainium2 NeuronCores.

Sharding: core c handles batch c//2 and heads 8*(c%2) .. 8*(c%2)+8 (half the
heads of one batch). Host sums the two half-head partial outputs per batch.

Per-core program (1 batch, 8 heads as 4 groups of 2), bf16 matmuls with fp32
PSUM accumulation:
  V:    V[tok, h*64+d] = x @ Wv directly (no PE transposes) -> vsb bf16,
        with a ones column per head (65th) for softmax denominators.
  A(g): Q^T/K^T [128 = 2 heads x (32 even | 32 odd), T] + RoPE
        (cos/sin muls on DVE, cross-term swap via SBUF DMAs).
  B(g): per q-chunk qc (512 q): causal S^T blocks [k=128, q<=512] per head
        (PE row-groups 0-63 / 64-127), causal mask via trib matmul on the
        diagonal staircase, ONE exp per kb on ACT -> pt bf16.
        AV per head: av_h [65, 512] = [values ; denominator] (ones column).
        Normalize: reciprocal (DVE) -> partition_broadcast (Pool) -> mul;
        odd head lands at partitions 0-63 and is DMA-shifted to attn rows
        64-127 (one [64,512] bf16 SBUF DMA per (g, qc)).
  C:    out[tok, :] accumulates attn_g^T @ Wo_g (K=128) over the 4 groups.
"""
import numpy as np
from contextlib import ExitStack

import concourse.bass as bass
import concourse.tile as tile
from concourse import bacc, mybir
from concourse.bass_utils import run_bass_kernel_spmd

F32 = mybir.dt.float32
BF16 = mybir.dt.bfloat16
AF = mybir.ActivationFunctionType

D, H, DK, T, B = 1024, 16, 64, 2048, 4
NCORES = 8
ROPE_THETA = 10000.0
_BUILT = {}


def _build_nc():
    nc = bacc.Bacc("TRN2", target_bir_lowering=False, debug=False,
                   num_devices=NCORES)
    xT = nc.dram_tensor("xT", [D, T], BF16, kind="ExternalInput").ap()
    wq = nc.dram_tensor("wq", [D, 512], BF16, kind="ExternalInput").ap()
    wk = nc.dram_tensor("wk", [D, 512], BF16, kind="ExternalInput").ap()
    wv = nc.dram_tensor("wv", [D, 512], BF16, kind="ExternalInput").ap()
    wo = nc.dram_tensor("wo", [512, D], BF16, kind="ExternalInput").ap()
    cc = nc.dram_tensor("cc", [128, T], BF16, kind="ExternalInput").ap()
    ss = nc.dram_tensor("ss", [128, T], BF16, kind="ExternalInput").ap()
    tri = nc.dram_tensor("tri", [128, 2, 128], BF16, kind="ExternalInput").ap()
    out = nc.dram_tensor("out", [T, D], BF16, kind="ExternalOutput").ap()

    x3 = xT.rearrange("(dt p) n -> p dt n", p=128)    # [128, 8, T]
    wq3 = wq.rearrange("(dt p) m -> p dt m", p=128)   # [128, 8, 512]
    wk3 = wk.rearrange("(dt p) m -> p dt m", p=128)
    wv3 = wv.rearrange("(dt p) m -> p dt m", p=128)
    wo3 = wo.rearrange("(g p) m -> p g m", p=128)     # [128, 4, 1024]

    with tile.TileContext(nc) as tc, ExitStack() as ctx:
        consts = ctx.enter_context(tc.tile_pool(name="consts", bufs=1))
        wpool = ctx.enter_context(tc.tile_pool(name="wpool", bufs=1))
        xin = ctx.enter_context(tc.tile_pool(name="xin", bufs=1))
        vtp = ctx.enter_context(tc.tile_pool(name="vtp", bufs=1))
        qkp = ctx.enter_context(tc.tile_pool(name="qkp", bufs=2))
        rope = ctx.enter_context(tc.tile_pool(name="rope", bufs=1))
        atp = ctx.enter_context(tc.tile_pool(name="atp", bufs=1))
        ptp = ctx.enter_context(tc.tile_pool(name="ptp", bufs=4))
        nrm = ctx.enter_context(tc.tile_pool(name="nrm", bufs=3))
        osb = ctx.enter_context(tc.tile_pool(name="osb", bufs=2))
        ps_mix = ctx.enter_context(tc.tile_pool(name="ps_mix", bufs=2, space="PSUM"))
        ps_s = ctx.enter_context(tc.tile_pool(name="ps_s", bufs=2, space="PSUM"))
        ps_av = ctx.enter_context(tc.tile_pool(name="ps_av", bufs=1, space="PSUM"))

        c_cc = consts.tile([128, T], BF16)
        c_ss = consts.tile([128, T], BF16)
        c_tri = consts.tile([128, 2, 128], BF16)
        w_q = wpool.tile([128, 8, 512], BF16)
        w_k = wpool.tile([128, 8, 512], BF16)
        w_v = wpool.tile([128, 8, 512], BF16)
        w_o = wpool.tile([128, 4, D], BF16)
        xt = xin.tile([128, 8, T], BF16)
        vsb = vtp.tile([128, 16, 8, 65], BF16)
        attn = atp.tile([128, 4, T], BF16)

        # V needs w_v + xt first; split per dt-chunk to parallelize queues
        # and let the first V matmuls start as soon as chunk 0 lands.
        # xt lands in per-dt 512-col quarters so V/A consume it as it streams
        for dt_i in range(8):
            nc.sync.dma_start(w_v[:, dt_i, :], wv3[:, dt_i, :])
        for tb in range(4):
            c0 = tb * 512
            for dt_i in range(8):
                nc.sync.dma_start(xt[:, dt_i, c0:c0 + 512],
                                  x3[:, dt_i, c0:c0 + 512])
        nc.sync.dma_start(w_q[:], wq3)
        nc.sync.dma_start(c_cc[:], cc)
        nc.sync.dma_start(c_ss[:], ss)
        nc.sync.dma_start(w_k[:], wk3)
        nc.sync.dma_start(c_tri[:], tri)
        nc.sync.dma_start(w_o[:], wo3)

        # ---------------- V: V[tok, h*64+d] = x @ Wv, written pre-transposed
        nc.gpsimd.memset(vsb[:, :, :, 64:65], 1.0)

        # Warm the PE clock (HAM) during the initial input-DMA wait: dummy
        # matmuls on the memset ones column need no DMA. Sized to end just
        # before the first real matmul's inputs land (~10us at ~270ns each)
        # so the HW clock gate is open (and stays open: trailing idle < the
        # 3.4us window).
        ones_col = vsb[:, 0, 0, 64:65]
        ones_row = vsb[:, :, :, 64:65].rearrange("p a b c -> p (a b c)")
        dav = ps_av.tile([128, 2, 512], F32, tag="av")
        for _ in range(30):
            nc.tensor.matmul(dav[0:1, 0, 0:128], ones_col, ones_row,
                             start=True, stop=True, skip_group_check=True)

        def v_quarter(tb):
            for tt in range(4 * tb, 4 * tb + 4):
                psV = ps_mix.tile([128, 512], F32, tag="mix")
                for dt_i in range(8):
                    nc.tensor.matmul(psV[:],
                                     xt[:, dt_i, tt * 128:(tt + 1) * 128],
                                     w_v[:, dt_i, :],
                                     start=(dt_i == 0), stop=(dt_i == 7))
                src = psV[:].rearrange("p (h d) -> p h d", h=8)
                nc.scalar.activation(vsb[:, tt, :, 0:64], src, AF.Copy)

        def phase_a(g, interleave=None):
            qt = qkp.tile([128, T], BF16, tag="qt")
            kt = qkp.tile([128, T], BF16, tag="kt")
            bq = rope.tile([128, T], BF16, tag="bq")
            bk = rope.tile([128, T], BF16, tag="bk")
            bsq = rope.tile([128, T], BF16, tag="bsq")
            bsk = rope.tile([128, T], BF16, tag="bsk")
            bwq = rope.tile([128, T], BF16, tag="bwq")
            bwk = rope.tile([128, T], BF16, tag="bwk")
            for tb in range(4):
                if interleave is not None:
                    interleave(tb)
                lt = tb * 512
                for wsb, bdst, bsdst, stg in ((w_q, bq, bsq, "sq"),
                                              (w_k, bk, bsk, "sk")):
                    psA = ps_mix.tile([128, 512], F32, tag="mix")
                    for dt_i in range(8):
                        nc.tensor.matmul(
                            psA[:], wsb[:, dt_i, g * 128:(g + 1) * 128],
                            xt[:, dt_i, lt:lt + 512],
                            start=(dt_i == 0), stop=(dt_i == 7))
                    # bf16 staging copy on ACT frees the PSUM slot quickly
                    # and lets both RoPE muls run in the DVE 4x mode.
                    stga = rope.tile([128, 512], BF16, tag=stg)
                    nc.vector.tensor_copy(stga[:], psA[:])
                    nc.vector.tensor_mul(bdst[:, lt:lt + 512], stga[:],
                                         c_cc[:, lt:lt + 512])
                    nc.vector.tensor_mul(bsdst[:, lt:lt + 512], stga[:],
                                         c_ss[:, lt:lt + 512])
                if tb in (1, 3):
                    hlo = (tb - 1) * 512
                    for bt, bw, bb, dest in ((bsq, bwq, bq, qt),
                                             (bsk, bwk, bk, kt)):
                        for hh in range(2):
                            r0 = hh * 64
                            nc.sync.dma_start(
                                bw[r0 + 32:r0 + 64, hlo:hlo + 1024],
                                bt[r0:r0 + 32, hlo:hlo + 1024])
                            nc.sync.dma_start(
                                bw[r0:r0 + 32, hlo:hlo + 1024],
                                bt[r0 + 32:r0 + 64, hlo:hlo + 1024])
                        nc.vector.tensor_add(dest[:, hlo:hlo + 1024],
                                             bb[:, hlo:hlo + 1024],
                                             bw[:, hlo:hlo + 1024])
            return qt, kt

        def phase_b_chunk(g, qt, kt, qc):
            qlo = qc * 512
            av = ps_av.tile([128, 2, 512], F32, tag="av")
            nkb = 4 * qc + 4

            def emit_av(kb, pt, qq, n, last):
                for h in range(2):
                    nc.tensor.matmul(av[0:65, h, qq:512],
                                     vsb[:, kb, 2 * g + h, :],
                                     pt[:, h, 0:n],
                                     start=(kb == 0), stop=last,
                                     skip_group_check=True)

            # Software-pipelined by one stage: av(kb) is emitted AFTER
            # scores(kb+1) so the in-order PE queue never stalls on exp(kb).
            pend = None
            for kb in range(nkb):
                k0 = kb * 128
                q0 = max(qlo, k0)
                n = qlo + 512 - q0
                qq = q0 - qlo
                diag = (q0 == k0)
                last = (kb == nkb - 1)
                sps = ps_s.tile([128, 2, 512], F32, tag="s")
                for h in range(2):
                    nc.tensor.matmul(
                        sps[:, h, 0:n],
                        kt[h * 64:(h + 1) * 64, k0:k0 + 128],
                        qt[h * 64:(h + 1) * 64, q0:qlo + 512],
                        start=True, stop=True,
                        skip_group_check=True)
                pt = ptp.tile([128, 2, 512], BF16, tag="pt")
                nc.scalar.activation(pt[:, :, 0:n], sps[:, :, 0:n], AF.Exp)
                if diag:
                    # causal mask: zero the upper triangle of the 128-wide
                    # diagonal block (all-bf16 SBUF mul -> DVE 4x mode)
                    nc.vector.tensor_mul(pt[:, :, 0:128], pt[:, :, 0:128],
                                         c_tri[:])
                if pend is not None:
                    emit_av(*pend)
                pend = (kb, pt, qq, n, last)
            emit_av(*pend)
            # Evacuate av banks ASAP (bf16 SBUF copies). Normalization:
            # fast approx reciprocal of the denominator row (f32, direct
            # from PSUM), then one SWDGE DMA that broadcasts it across 64
            # partitions AND casts f32->bf16, then bf16 DVE muls (2x mode).
            avc = nrm.tile([128, 2, 512], BF16, tag="avc")
            dens = nrm.tile([1, 2, 512], F32, tag="dens")
            rec = nrm.tile([1, 2, 512], F32, tag="rec")
            recb = nrm.tile([1, 2, 512], BF16, tag="recb")
            rbs = nrm.tile([64, 2, 512], BF16, tag="rbs")
            tmp = nrm.tile([64, 512], BF16, tag="tmp")
            nc.scalar.activation(avc[0:65, 0, :], av[0:65, 0, :], AF.Copy)
            nc.vector.tensor_copy(avc[0:65, 1, :], av[0:65, 1, :])
            # custom-DVE ops read SBUF; stage the PSUM denominator row first
            # (on ACT, which has idle headroom — DVE is busier)
            nc.scalar.activation(dens[0:1, :, :], av[64:65, :, :], AF.Copy)
            nc.vector.reciprocal_approx_fast(rec[0:1, :, :], dens[0:1, :, :])
            with nc.allow_low_precision(reason="1/denom in bf16: 0.4% uniform row scale, well within tolerance"):
                nc.vector.tensor_copy(recb[0:1, :, :], rec[0:1, :, :])
            nc.gpsimd.partition_broadcast(rbs[0:64, :, :], recb[0:1, :, :])
            nc.vector.tensor_mul(attn[0:64, g, qlo:qlo + 512],
                                 avc[0:64, 0, :], rbs[0:64, 0, :])
            nc.vector.tensor_mul(tmp[0:64, :], avc[0:64, 1, :],
                                 rbs[0:64, 1, :])
            nc.sync.dma_start(attn[64:128, g, qlo:qlo + 512], tmp[0:64, :])

        def phase_c(qc):
            for tp2 in range(2 * qc, 2 * qc + 2):
                o_sb = osb.tile([128, 2, D], BF16, tag="osb")
                for s in range(2):
                    tt = tp2 * 2 + s
                    for h5 in range(2):
                        psO = ps_mix.tile([128, 512], F32, tag="mix")
                        for g in range(4):
                            nc.tensor.matmul(
                                psO[:], attn[:, g, tt * 128:(tt + 1) * 128],
                                w_o[:, g, h5 * 512:(h5 + 1) * 512],
                                start=(g == 0), stop=(g == 3))
                        dst = o_sb[:, s, h5 * 512:(h5 + 1) * 512]
                        # split PSUM evacuation between ACT and DVE so
                        # neither engine becomes the bottleneck
                        if s == 0:
                            nc.scalar.activation(dst, psO[:], AF.Copy)
                        else:
                            nc.vector.tensor_copy(dst, psO[:])
                row = tp2 * 256
                nc.sync.dma_start(
                    out[row:row + 256, :].rearrange("(s p) f -> p s f", p=128),
                    o_sb[:])

        # Emission order: A(g+1) interleaves with B(g) so the PE gap-fills;
        # C(qc) right after B(3, qc) — all four groups' attn cols are ready.
        for tb in range(4):
            v_quarter(tb)
        qk = {0: phase_a(0)}
        for g in range(4):
            # g=3 runs qc=0 first (smallest chunk: nothing can fill the PE
            # while it waits on B(3,0)), then descending so C(qc) work is
            # always available to gap-fill the remaining chunks. C(0) is
            # held back to the very end: its deps are ready long before,
            # so it fills the PE while the last chunk's normalize tail
            # (recip -> broadcast -> mul) finishes.
            qcs = (0, 3, 2, 1) if g == 3 else range(4)
            for qc in qcs:
                phase_b_chunk(g, *qk[g], qc)
                if g == 3 and qc != 0:
                    phase_c(qc)
            if g + 1 <= 3:
                qk[g + 1] = phase_a(g + 1)
        phase_c(0)

    nc.compile()
    return nc


def _host_prep(x, W_qkv, W_o, token_positions):
    import ml_dtypes
    bf = ml_dtypes.bfloat16
    x = np.asarray(x, np.float32)
    W_qkv = np.asarray(W_qkv, np.float32)
    W_o = np.asarray(W_o, np.float32)
    pos = np.asarray(token_positions, np.float64)
    i = np.arange(32)
    inv = 1.0 / (ROPE_THETA ** (2 * i / DK))
    ang = pos[None, :] * inv[:, None]
    CC = np.tile(np.cos(ang), (4, 1)).astype(bf)
    sn = np.sin(ang)
    SS = np.concatenate([sn, -sn, sn, -sn], 0).astype(bf)
    tri01 = np.where(np.arange(128)[:, None] <= np.arange(128)[None, :],
                     1.0, 0.0).astype(bf)
    tri2 = np.repeat(tri01[:, None, :], 2, axis=1)
    xTb = [np.ascontiguousarray(x[b].T).astype(bf) for b in range(B)]
    in_maps = []
    for c in range(NCORES):
        b, hg = c // 2, c % 2
        qcols, vcols = [], []
        for h in range(hg * 8, hg * 8 + 8):
            for half in range(2):
                qcols.extend(h * DK + 2 * ii + half for ii in range(32))
            vcols.extend(h * DK + d for d in range(DK))
        qcols = np.array(qcols)
        vcols = np.array(vcols)
        in_maps.append({
            "xT": xTb[b],
            "wq": np.ascontiguousarray(W_qkv[:, 0 * D + qcols]).astype(bf),
            "wk": np.ascontiguousarray(W_qkv[:, 1 * D + qcols] / 8.0).astype(bf),
            "wv": np.ascontiguousarray(W_qkv[:, 2 * D + vcols]).astype(bf),
            "wo": np.ascontiguousarray(W_o[vcols, :]).astype(bf),
            "cc": CC, "ss": SS, "tri": tri2,
        })
    return in_maps


def kernel(x, W_qkv, W_o, token_positions, _trace=False):
    in_maps = _host_prep(x, W_qkv, W_o, token_positions)
    if "nc" not in _BUILT:
        _BUILT["nc"] = _build_nc()
    res = run_bass_kernel_spmd(_BUILT["nc"], in_maps,
                               core_ids=list(range(NCORES)), trace=_trace)
    _BUILT["last_result"] = res
    total = np.zeros((B, T, D), np.float32)
    for c in range(NCORES):
        total[c // 2] += np.asarray(res.results[c]["out"], np.float32)
    return total



# revision 18
# speedup vs baseline: 1.3063x; 1.0057x over previous
"""Causal multi-head self-attention with RoPE on 8 Trainium2 NeuronCores.

Sharding: core c handles batch c//2 and heads 8*(c%2) .. 8*(c%2)+8 (half the
heads of one batch). Host sums the two half-head partial outputs per batch.

Per-core program (1 batch, 8 heads as 4 groups of 2), bf16 matmuls with fp32
PSUM accumulation:
  V:    V[tok, h*64+d] = x @ Wv directly (no PE transposes) -> vsb bf16,
        with a ones column per head (65th) for softmax denominators.
  A(g): Q^T/K^T [128 = 2 heads x (32 even | 32 odd), T] + RoPE
        (cos/sin muls on DVE, cross-term swap via SBUF DMAs).
  B(g): per q-chunk qc (512 q): causal S^T blocks [k=128, q<=512] per head
        (PE row-groups 0-63 / 64-127), causal mask via trib matmul on the
        diagonal staircase, ONE exp per kb on ACT -> pt bf16.
        AV per head: av_h [65, 512] = [values ; denominator] (ones column).
        Normalize: reciprocal (DVE) -> partition_broadcast (Pool) -> mul;
        odd head lands at partitions 0-63 and is DMA-shifted to attn rows
        64-127 (one [64,512] bf16 SBUF DMA per (g, qc)).
  C:    out[tok, :] accumulates attn_g^T @ Wo_g (K=128) over the 4 groups.
"""
import numpy as np
from contextlib import ExitStack

import concourse.bass as bass
import concourse.tile as tile
from concourse import bacc, mybir
from concourse.bass_utils import run_bass_kernel_spmd

F32 = mybir.dt.float32
BF16 = mybir.dt.bfloat16
AF = mybir.ActivationFunctionType

D, H, DK, T, B = 1024, 16, 64, 2048, 4
NCORES = 8
ROPE_THETA = 10000.0
_BUILT = {}


def _build_nc():
    nc = bacc.Bacc("TRN2", target_bir_lowering=False, debug=False,
                   num_devices=NCORES)
    xT = nc.dram_tensor("xT", [D, T], BF16, kind="ExternalInput").ap()
    wq = nc.dram_tensor("wq", [D, 512], BF16, kind="ExternalInput").ap()
    wk = nc.dram_tensor("wk", [D, 512], BF16, kind="ExternalInput").ap()
    wv = nc.dram_tensor("wv", [D, 512], BF16, kind="ExternalInput").ap()
    wo = nc.dram_tensor("wo", [512, D], BF16, kind="ExternalInput").ap()
    cc = nc.dram_tensor("cc", [128, T], BF16, kind="ExternalInput").ap()
    ss = nc.dram_tensor("ss", [128, T], BF16, kind="ExternalInput").ap()
    tri = nc.dram_tensor("tri", [128, 2, 128], BF16, kind="ExternalInput").ap()
    out = nc.dram_tensor("out", [T, D], BF16, kind="ExternalOutput").ap()

    x3 = xT.rearrange("(dt p) n -> p dt n", p=128)    # [128, 8, T]
    wq3 = wq.rearrange("(dt p) m -> p dt m", p=128)   # [128, 8, 512]
    wk3 = wk.rearrange("(dt p) m -> p dt m", p=128)
    wv3 = wv.rearrange("(dt p) m -> p dt m", p=128)
    wo3 = wo.rearrange("(g p) m -> p g m", p=128)     # [128, 4, 1024]

    with tile.TileContext(nc) as tc, ExitStack() as ctx:
        consts = ctx.enter_context(tc.tile_pool(name="consts", bufs=1))
        wpool = ctx.enter_context(tc.tile_pool(name="wpool", bufs=1))
        xin = ctx.enter_context(tc.tile_pool(name="xin", bufs=1))
        vtp = ctx.enter_context(tc.tile_pool(name="vtp", bufs=1))
        qkp = ctx.enter_context(tc.tile_pool(name="qkp", bufs=2))
        rope = ctx.enter_context(tc.tile_pool(name="rope", bufs=1))
        atp = ctx.enter_context(tc.tile_pool(name="atp", bufs=1))
        ptp = ctx.enter_context(tc.tile_pool(name="ptp", bufs=4))
        nrm = ctx.enter_context(tc.tile_pool(name="nrm", bufs=3))
        osb = ctx.enter_context(tc.tile_pool(name="osb", bufs=2))
        ps_mix = ctx.enter_context(tc.tile_pool(name="ps_mix", bufs=2, space="PSUM"))
        ps_s = ctx.enter_context(tc.tile_pool(name="ps_s", bufs=2, space="PSUM"))
        ps_av = ctx.enter_context(tc.tile_pool(name="ps_av", bufs=1, space="PSUM"))

        c_cc = consts.tile([128, T], BF16)
        c_ss = consts.tile([128, T], BF16)
        c_tri = consts.tile([128, 2, 128], BF16)
        w_q = wpool.tile([128, 8, 512], BF16)
        w_k = wpool.tile([128, 8, 512], BF16)
        w_v = wpool.tile([128, 8, 512], BF16)
        w_o = wpool.tile([128, 4, D], BF16)
        xt = xin.tile([128, 8, T], BF16)
        vsb = vtp.tile([128, 16, 8, 65], BF16)
        attn = atp.tile([128, 4, T], BF16)

        # V needs w_v + xt first; interleave per-dt (w_v[dt], xt[dt, tb0])
        # pairs so the first V matmul chain can start streaming as soon as
        # the first pairs land, then the rest of xt in 512-col quarters.
        for dt_i in range(8):
            nc.sync.dma_start(w_v[:, dt_i, :], wv3[:, dt_i, :])
            nc.sync.dma_start(xt[:, dt_i, 0:512], x3[:, dt_i, 0:512])
        for tb in range(1, 4):
            c0 = tb * 512
            for dt_i in range(8):
                nc.sync.dma_start(xt[:, dt_i, c0:c0 + 512],
                                  x3[:, dt_i, c0:c0 + 512])
        nc.sync.dma_start(w_q[:], wq3)
        nc.sync.dma_start(c_cc[:], cc)
        nc.sync.dma_start(c_ss[:], ss)
        nc.sync.dma_start(w_k[:], wk3)
        nc.sync.dma_start(c_tri[:], tri)
        nc.sync.dma_start(w_o[:], wo3)

        # ---------------- V: V[tok, h*64+d] = x @ Wv, written pre-transposed
        nc.gpsimd.memset(vsb[:, :, :, 64:65], 1.0)

        # Warm the PE clock (HAM) during the initial input-DMA wait: dummy
        # matmuls on the memset ones column need no DMA. Sized to end just
        # before the first real matmul's inputs land (~10us at ~270ns each)
        # so the HW clock gate is open (and stays open: trailing idle < the
        # 3.4us window).
        ones_col = vsb[:, 0, 0, 64:65]
        ones_row = vsb[:, :, :, 64:65].rearrange("p a b c -> p (a b c)")
        dav = ps_av.tile([128, 2, 512], F32, tag="av")
        for _ in range(24):
            nc.tensor.matmul(dav[0:1, 0, 0:128], ones_col, ones_row,
                             start=True, stop=True, skip_group_check=True)

        def v_quarter(tb):
            for tt in range(4 * tb, 4 * tb + 4):
                psV = ps_mix.tile([128, 512], F32, tag="mix")
                for dt_i in range(8):
                    nc.tensor.matmul(psV[:],
                                     xt[:, dt_i, tt * 128:(tt + 1) * 128],
                                     w_v[:, dt_i, :],
                                     start=(dt_i == 0), stop=(dt_i == 7))
                src = psV[:].rearrange("p (h d) -> p h d", h=8)
                nc.scalar.activation(vsb[:, tt, :, 0:64], src, AF.Copy)

        def phase_a(g, interleave=None):
            qt = qkp.tile([128, T], BF16, tag="qt")
            kt = qkp.tile([128, T], BF16, tag="kt")
            bq = rope.tile([128, T], BF16, tag="bq")
            bk = rope.tile([128, T], BF16, tag="bk")
            bsq = rope.tile([128, T], BF16, tag="bsq")
            bsk = rope.tile([128, T], BF16, tag="bsk")
            bwq = rope.tile([128, T], BF16, tag="bwq")
            bwk = rope.tile([128, T], BF16, tag="bwk")
            for tb in range(4):
                if interleave is not None:
                    interleave(tb)
                lt = tb * 512
                for wsb, bdst, bsdst, stg in ((w_q, bq, bsq, "sq"),
                                              (w_k, bk, bsk, "sk")):
                    psA = ps_mix.tile([128, 512], F32, tag="mix")
                    for dt_i in range(8):
                        nc.tensor.matmul(
                            psA[:], wsb[:, dt_i, g * 128:(g + 1) * 128],
                            xt[:, dt_i, lt:lt + 512],
                            start=(dt_i == 0), stop=(dt_i == 7))
                    # bf16 staging copy on ACT frees the PSUM slot quickly
                    # and lets both RoPE muls run in the DVE 4x mode.
                    stga = rope.tile([128, 512], BF16, tag=stg)
                    nc.vector.tensor_copy(stga[:], psA[:])
                    nc.vector.tensor_mul(bdst[:, lt:lt + 512], stga[:],
                                         c_cc[:, lt:lt + 512])
                    nc.vector.tensor_mul(bsdst[:, lt:lt + 512], stga[:],
                                         c_ss[:, lt:lt + 512])
                if tb in (1, 3):
                    hlo = (tb - 1) * 512
                    for bt, bw, bb, dest in ((bsq, bwq, bq, qt),
                                             (bsk, bwk, bk, kt)):
                        for hh in range(2):
                            r0 = hh * 64
                            nc.sync.dma_start(
                                bw[r0 + 32:r0 + 64, hlo:hlo + 1024],
                                bt[r0:r0 + 32, hlo:hlo + 1024])
                            nc.sync.dma_start(
                                bw[r0:r0 + 32, hlo:hlo + 1024],
                                bt[r0 + 32:r0 + 64, hlo:hlo + 1024])
                        nc.vector.tensor_add(dest[:, hlo:hlo + 1024],
                                             bb[:, hlo:hlo + 1024],
                                             bw[:, hlo:hlo + 1024])
            return qt, kt

        def phase_b_chunk(g, qt, kt, qc):
            qlo = qc * 512
            av = ps_av.tile([128, 2, 512], F32, tag="av")
            nkb = 4 * qc + 4

            def emit_av(kb, pt, qq, n, last):
                for h in range(2):
                    nc.tensor.matmul(av[0:65, h, qq:512],
                                     vsb[:, kb, 2 * g + h, :],
                                     pt[:, h, 0:n],
                                     start=(kb == 0), stop=last,
                                     skip_group_check=True)

            # Software-pipelined by one stage: av(kb) is emitted AFTER
            # scores(kb+1) so the in-order PE queue never stalls on exp(kb).
            pend = None
            for kb in range(nkb):
                k0 = kb * 128
                q0 = max(qlo, k0)
                n = qlo + 512 - q0
                qq = q0 - qlo
                diag = (q0 == k0)
                last = (kb == nkb - 1)
                sps = ps_s.tile([128, 2, 512], F32, tag="s")
                for h in range(2):
                    nc.tensor.matmul(
                        sps[:, h, 0:n],
                        kt[h * 64:(h + 1) * 64, k0:k0 + 128],
                        qt[h * 64:(h + 1) * 64, q0:qlo + 512],
                        start=True, stop=True,
                        skip_group_check=True)
                pt = ptp.tile([128, 2, 512], BF16, tag="pt")
                nc.scalar.activation(pt[:, :, 0:n], sps[:, :, 0:n], AF.Exp)
                if diag:
                    # causal mask: zero the upper triangle of the 128-wide
                    # diagonal block (all-bf16 SBUF mul -> DVE 4x mode)
                    nc.vector.tensor_mul(pt[:, :, 0:128], pt[:, :, 0:128],
                                         c_tri[:])
                if pend is not None:
                    emit_av(*pend)
                pend = (kb, pt, qq, n, last)
            emit_av(*pend)
            # Evacuate av banks ASAP (bf16 SBUF copies). Normalization:
            # fast approx reciprocal of the denominator row (f32, direct
            # from PSUM), then one SWDGE DMA that broadcasts it across 64
            # partitions AND casts f32->bf16, then bf16 DVE muls (2x mode).
            avc = nrm.tile([128, 2, 512], BF16, tag="avc")
            dens = nrm.tile([1, 2, 512], F32, tag="dens")
            rec = nrm.tile([1, 2, 512], F32, tag="rec")
            recb = nrm.tile([1, 2, 512], BF16, tag="recb")
            rbs = nrm.tile([64, 2, 512], BF16, tag="rbs")
            tmp = nrm.tile([64, 512], BF16, tag="tmp")
            # custom-DVE ops read SBUF; stage the PSUM denominator row first,
            # split across ACT/DVE so the av banks free up ~0.5us sooner and
            # the reciprocal chain starts earlier.
            nc.scalar.activation(dens[0:1, 0, :], av[64:65, 0, :], AF.Copy)
            nc.vector.tensor_copy(dens[0:1, 1, :], av[64:65, 1, :])
            nc.scalar.activation(avc[0:65, 0, :], av[0:65, 0, :], AF.Copy)
            nc.vector.tensor_copy(avc[0:65, 1, :], av[0:65, 1, :])
            nc.vector.reciprocal_approx_fast(rec[0:1, :, :], dens[0:1, :, :])
            with nc.allow_low_precision(reason="1/denom in bf16: 0.4% uniform row scale, well within tolerance"):
                nc.vector.tensor_copy(recb[0:1, :, :], rec[0:1, :, :])
            nc.gpsimd.partition_broadcast(rbs[0:64, :, :], recb[0:1, :, :])
            nc.vector.tensor_mul(attn[0:64, g, qlo:qlo + 512],
                                 avc[0:64, 0, :], rbs[0:64, 0, :])
            nc.vector.tensor_mul(tmp[0:64, :], avc[0:64, 1, :],
                                 rbs[0:64, 1, :])
            nc.sync.dma_start(attn[64:128, g, qlo:qlo + 512], tmp[0:64, :])

        def phase_c(qc):
            for tp2 in range(2 * qc, 2 * qc + 2):
                o_sb = osb.tile([128, 2, D], BF16, tag="osb")
                for s in range(2):
                    tt = tp2 * 2 + s
                    for h5 in range(2):
                        psO = ps_mix.tile([128, 512], F32, tag="mix")
                        for g in range(4):
                            nc.tensor.matmul(
                                psO[:], attn[:, g, tt * 128:(tt + 1) * 128],
                                w_o[:, g, h5 * 512:(h5 + 1) * 512],
                                start=(g == 0), stop=(g == 3))
                        dst = o_sb[:, s, h5 * 512:(h5 + 1) * 512]
                        # split PSUM evacuation between ACT and DVE so
                        # neither engine becomes the bottleneck
                        if s == 0:
                            nc.scalar.activation(dst, psO[:], AF.Copy)
                        else:
                            nc.vector.tensor_copy(dst, psO[:])
                row = tp2 * 256
                nc.sync.dma_start(
                    out[row:row + 256, :].rearrange("(s p) f -> p s f", p=128),
                    o_sb[:])

        # Emission order: A(g+1) interleaves with B(g) so the PE gap-fills;
        # C(qc) right after B(3, qc) — all four groups' attn cols are ready.
        for tb in range(4):
            v_quarter(tb)
        qk = {0: phase_a(0)}
        for g in range(4):
            # g=3 runs qc=0 first (smallest chunk: nothing can fill the PE
            # while it waits on B(3,0)), then descending so C(qc) work is
            # always available to gap-fill the remaining chunks. C(0) is
            # held back to the very end: its deps are ready long before,
            # so it fills the PE while the last chunk's normalize tail
            # (recip -> broadcast -> mul) finishes.
            qcs = (0, 3, 2, 1) if g == 3 else range(4)
            for qc in qcs:
                phase_b_chunk(g, *qk[g], qc)
                if g == 3 and qc in (3, 2):
                    phase_c(qc)
            if g + 1 <= 3:
                qk[g + 1] = phase_a(g + 1)
        # C(0)'s inputs were ready long ago: emit it between B(3,1) and
        # C(1) so the in-order PE queue has work while the last chunk's
        # normalize tail (recip -> broadcast -> mul -> shift) completes.
        phase_c(0)
        phase_c(1)

    nc.compile()
    return nc


def _host_prep(x, W_qkv, W_o, token_positions):
    import ml_dtypes
    bf = ml_dtypes.bfloat16
    x = np.asarray(x, np.float32)
    W_qkv = np.asarray(W_qkv, np.float32)
    W_o = np.asarray(W_o, np.float32)
    pos = np.asarray(token_positions, np.float64)
    i = np.arange(32)
    inv = 1.0 / (ROPE_THETA ** (2 * i / DK))
    ang = pos[None, :] * inv[:, None]
    CC = np.tile(np.cos(ang), (4, 1)).astype(bf)
    sn = np.sin(ang)
    SS = np.concatenate([sn, -sn, sn, -sn], 0).astype(bf)
    tri01 = np.where(np.arange(128)[:, None] <= np.arange(128)[None, :],
                     1.0, 0.0).astype(bf)
    tri2 = np.repeat(tri01[:, None, :], 2, axis=1)
    xTb = [np.ascontiguousarray(x[b].T).astype(bf) for b in range(B)]
    in_maps = []
    for c in range(NCORES):
        b, hg = c // 2, c % 2
        qcols, vcols = [], []
        for h in range(hg * 8, hg * 8 + 8):
            for half in range(2):
                qcols.extend(h * DK + 2 * ii + half for ii in range(32))
            vcols.extend(h * DK + d for d in range(DK))
        qcols = np.array(qcols)
        vcols = np.array(vcols)
        in_maps.append({
            "xT": xTb[b],
            "wq": np.ascontiguousarray(W_qkv[:, 0 * D + qcols]).astype(bf),
            "wk": np.ascontiguousarray(W_qkv[:, 1 * D + qcols] / 8.0).astype(bf),
            "wv": np.ascontiguousarray(W_qkv[:, 2 * D + vcols]).astype(bf),
            "wo": np.ascontiguousarray(W_o[vcols, :]).astype(bf),
            "cc": CC, "ss": SS, "tri": tri2,
        })
    return in_maps


def kernel(x, W_qkv, W_o, token_positions, _trace=False):
    in_maps = _host_prep(x, W_qkv, W_o, token_positions)
    if "nc" not in _BUILT:
        _BUILT["nc"] = _build_nc()
    res = run_bass_kernel_spmd(_BUILT["nc"], in_maps,
                               core_ids=list(range(NCORES)), trace=_trace)
    _BUILT["last_result"] = res
    total = np.zeros((B, T, D), np.float32)
    for c in range(NCORES):
        total[c // 2] += np.asarray(res.results[c]["out"], np.float32)
    return total



# revision 27
# speedup vs baseline: 1.3075x; 1.0009x over previous
"""Causal multi-head self-attention with RoPE on 8 Trainium2 NeuronCores.

Sharding: core c handles batch c//2 and heads 8*(c%2) .. 8*(c%2)+8 (half the
heads of one batch). Host sums the two half-head partial outputs per batch.

Per-core program (1 batch, 8 heads as 4 groups of 2), bf16 matmuls with fp32
PSUM accumulation:
  V:    V[tok, h*64+d] = x @ Wv directly (no PE transposes) -> vsb bf16,
        with a ones column per head (65th) for softmax denominators.
  A(g): Q^T/K^T [128 = 2 heads x (32 even | 32 odd), T] + RoPE
        (cos/sin muls on DVE, cross-term swap via SBUF DMAs).
  B(g): per q-chunk qc (512 q): causal S^T blocks [k=128, q<=512] per head
        (PE row-groups 0-63 / 64-127), causal mask via trib matmul on the
        diagonal staircase, ONE exp per kb on ACT -> pt bf16.
        AV per head: av_h [65, 512] = [values ; denominator] (ones column).
        Normalize: reciprocal (DVE) -> partition_broadcast (Pool) -> mul;
        odd head lands at partitions 0-63 and is DMA-shifted to attn rows
        64-127 (one [64,512] bf16 SBUF DMA per (g, qc)).
  C:    out[tok, :] accumulates attn_g^T @ Wo_g (K=128) over the 4 groups.
"""
import numpy as np
from contextlib import ExitStack

import concourse.bass as bass
import concourse.tile as tile
from concourse import bacc, mybir
from concourse.bass_utils import run_bass_kernel_spmd

F32 = mybir.dt.float32
BF16 = mybir.dt.bfloat16
AF = mybir.ActivationFunctionType

D, H, DK, T, B = 1024, 16, 64, 2048, 4
NCORES = 8
ROPE_THETA = 10000.0
_BUILT = {}


def _build_nc():
    nc = bacc.Bacc("TRN2", target_bir_lowering=False, debug=False,
                   num_devices=NCORES)
    xT = nc.dram_tensor("xT", [D, T], BF16, kind="ExternalInput").ap()
    wq = nc.dram_tensor("wq", [D, 512], BF16, kind="ExternalInput").ap()
    wk = nc.dram_tensor("wk", [D, 512], BF16, kind="ExternalInput").ap()
    wv = nc.dram_tensor("wv", [D, 512], BF16, kind="ExternalInput").ap()
    wo = nc.dram_tensor("wo", [512, D], BF16, kind="ExternalInput").ap()
    cc = nc.dram_tensor("cc", [128, T], BF16, kind="ExternalInput").ap()
    ss = nc.dram_tensor("ss", [128, T], BF16, kind="ExternalInput").ap()
    tri = nc.dram_tensor("tri", [128, 2, 128], BF16, kind="ExternalInput").ap()
    out = nc.dram_tensor("out", [T, D], BF16, kind="ExternalOutput").ap()

    x3 = xT.rearrange("(dt p) n -> p dt n", p=128)    # [128, 8, T]
    wq3 = wq.rearrange("(dt p) m -> p dt m", p=128)   # [128, 8, 512]
    wk3 = wk.rearrange("(dt p) m -> p dt m", p=128)
    wv3 = wv.rearrange("(dt p) m -> p dt m", p=128)
    wo3 = wo.rearrange("(g p) m -> p g m", p=128)     # [128, 4, 1024]

    with tile.TileContext(nc) as tc, ExitStack() as ctx:
        consts = ctx.enter_context(tc.tile_pool(name="consts", bufs=1))
        wpool = ctx.enter_context(tc.tile_pool(name="wpool", bufs=1))
        xin = ctx.enter_context(tc.tile_pool(name="xin", bufs=1))
        vtp = ctx.enter_context(tc.tile_pool(name="vtp", bufs=1))
        qkp = ctx.enter_context(tc.tile_pool(name="qkp", bufs=2))
        rope = ctx.enter_context(tc.tile_pool(name="rope", bufs=1))
        atp = ctx.enter_context(tc.tile_pool(name="atp", bufs=1))
        ptp = ctx.enter_context(tc.tile_pool(name="ptp", bufs=4))
        nrm = ctx.enter_context(tc.tile_pool(name="nrm", bufs=3))
        osb = ctx.enter_context(tc.tile_pool(name="osb", bufs=2))
        ps_mix = ctx.enter_context(tc.tile_pool(name="ps_mix", bufs=2, space="PSUM"))
        ps_s = ctx.enter_context(tc.tile_pool(name="ps_s", bufs=2, space="PSUM"))
        ps_av = ctx.enter_context(tc.tile_pool(name="ps_av", bufs=1, space="PSUM"))

        c_cc = consts.tile([128, T], BF16)
        c_ss = consts.tile([128, T], BF16)
        c_tri = consts.tile([128, 2, 128], BF16)
        w_q = wpool.tile([128, 8, 512], BF16)
        w_k = wpool.tile([128, 8, 512], BF16)
        w_v = wpool.tile([128, 8, 512], BF16)
        w_o = wpool.tile([128, 4, D], BF16)
        xt = xin.tile([128, 8, T], BF16)
        vsb = vtp.tile([128, 16, 8, 65], BF16)
        attn = atp.tile([128, 4, T], BF16)

        # V needs w_v + xt first; interleave per-dt (w_v[dt], xt[dt, tb0])
        # pairs so the first V matmul chain can start streaming as soon as
        # the first pairs land, then the rest of xt in 512-col quarters.
        for dt_i in range(8):
            nc.sync.dma_start(w_v[:, dt_i, :], wv3[:, dt_i, :])
            nc.sync.dma_start(xt[:, dt_i, 0:512], x3[:, dt_i, 0:512])
        for tb in range(1, 4):
            c0 = tb * 512
            for dt_i in range(8):
                nc.sync.dma_start(xt[:, dt_i, c0:c0 + 512],
                                  x3[:, dt_i, c0:c0 + 512])
        nc.sync.dma_start(w_q[:], wq3)
        nc.sync.dma_start(c_cc[:], cc)
        nc.sync.dma_start(c_ss[:], ss)
        nc.sync.dma_start(w_k[:], wk3)
        nc.sync.dma_start(c_tri[:], tri)
        nc.sync.dma_start(w_o[:], wo3)

        # ---------------- V: V[tok, h*64+d] = x @ Wv, written pre-transposed
        nc.gpsimd.memset(vsb[:, :, :, 64:65], 1.0)

        # Warm the PE clock (HAM) during the initial input-DMA wait: dummy
        # matmuls on the memset ones column need no DMA. Sized to end just
        # before the first real matmul's inputs land (~10us at ~270ns each)
        # so the HW clock gate is open (and stays open: trailing idle < the
        # 3.4us window).
        ones_col = vsb[:, 0, 0, 64:65]
        ones_row = vsb[:, :, :, 64:65].rearrange("p a b c -> p (a b c)")
        dav = ps_av.tile([128, 2, 512], F32, tag="av")
        for _ in range(24):
            nc.tensor.matmul(dav[0:1, 0, 0:128], ones_col, ones_row,
                             start=True, stop=True, skip_group_check=True)

        def v_quarter(tb):
            for tt in range(4 * tb, 4 * tb + 4):
                psV = ps_mix.tile([128, 512], F32, tag="mix")
                for dt_i in range(8):
                    nc.tensor.matmul(psV[:],
                                     xt[:, dt_i, tt * 128:(tt + 1) * 128],
                                     w_v[:, dt_i, :],
                                     start=(dt_i == 0), stop=(dt_i == 7))
                src = psV[:].rearrange("p (h d) -> p h d", h=8)
                nc.scalar.activation(vsb[:, tt, :, 0:64], src, AF.Copy)

        def phase_a(g, interleave=None):
            qt = qkp.tile([128, T], BF16, tag="qt")
            kt = qkp.tile([128, T], BF16, tag="kt")
            bq = rope.tile([128, T], BF16, tag="bq")
            bk = rope.tile([128, T], BF16, tag="bk")
            bsq = rope.tile([128, T], BF16, tag="bsq")
            bsk = rope.tile([128, T], BF16, tag="bsk")
            bwq = rope.tile([128, T], BF16, tag="bwq")
            bwk = rope.tile([128, T], BF16, tag="bwk")
            for tb in range(4):
                if interleave is not None:
                    interleave(tb)
                lt = tb * 512
                for wsb, bdst, bsdst, stg in ((w_q, bq, bsq, "sq"),
                                              (w_k, bk, bsk, "sk")):
                    psA = ps_mix.tile([128, 512], F32, tag="mix")
                    for dt_i in range(8):
                        nc.tensor.matmul(
                            psA[:], wsb[:, dt_i, g * 128:(g + 1) * 128],
                            xt[:, dt_i, lt:lt + 512],
                            start=(dt_i == 0), stop=(dt_i == 7))
                    # bf16 staging copy on ACT frees the PSUM slot quickly
                    # and lets both RoPE muls run in the DVE 4x mode.
                    stga = rope.tile([128, 512], BF16, tag=stg)
                    nc.vector.tensor_copy(stga[:], psA[:])
                    nc.vector.tensor_mul(bdst[:, lt:lt + 512], stga[:],
                                         c_cc[:, lt:lt + 512])
                    nc.vector.tensor_mul(bsdst[:, lt:lt + 512], stga[:],
                                         c_ss[:, lt:lt + 512])
                if tb in (1, 3):
                    hlo = (tb - 1) * 512
                    for bt, bw, bb, dest in ((bsq, bwq, bq, qt),
                                             (bsk, bwk, bk, kt)):
                        for hh in range(2):
                            r0 = hh * 64
                            nc.sync.dma_start(
                                bw[r0 + 32:r0 + 64, hlo:hlo + 1024],
                                bt[r0:r0 + 32, hlo:hlo + 1024])
                            nc.sync.dma_start(
                                bw[r0:r0 + 32, hlo:hlo + 1024],
                                bt[r0 + 32:r0 + 64, hlo:hlo + 1024])
                        nc.vector.tensor_add(dest[:, hlo:hlo + 1024],
                                             bb[:, hlo:hlo + 1024],
                                             bw[:, hlo:hlo + 1024])
            return qt, kt

        def phase_b_chunk(g, qt, kt, qc, fill=None):
            qlo = qc * 512
            av = ps_av.tile([128, 2, 512], F32, tag="av")
            nkb = 4 * qc + 4

            def emit_av(kb, pt, qq, n, last):
                for h in range(2):
                    nc.tensor.matmul(av[0:65, h, qq:512],
                                     vsb[:, kb, 2 * g + h, :],
                                     pt[:, h, 0:n],
                                     start=(kb == 0), stop=last,
                                     skip_group_check=True)

            # Software-pipelined by one stage: av(kb) is emitted AFTER
            # scores(kb+1) so the in-order PE queue never stalls on exp(kb).
            pend = None
            for kb in range(nkb):
                k0 = kb * 128
                q0 = max(qlo, k0)
                n = qlo + 512 - q0
                qq = q0 - qlo
                diag = (q0 == k0)
                last = (kb == nkb - 1)
                sps = ps_s.tile([128, 2, 512], F32, tag="s")
                for h in range(2):
                    nc.tensor.matmul(
                        sps[:, h, 0:n],
                        kt[h * 64:(h + 1) * 64, k0:k0 + 128],
                        qt[h * 64:(h + 1) * 64, q0:qlo + 512],
                        start=True, stop=True,
                        skip_group_check=True)
                pt = ptp.tile([128, 2, 512], BF16, tag="pt")
                nc.scalar.activation(pt[:, :, 0:n], sps[:, :, 0:n], AF.Exp)
                if diag:
                    # causal mask: zero the upper triangle of the 128-wide
                    # diagonal block (all-bf16 SBUF mul -> DVE 4x mode)
                    nc.vector.tensor_mul(pt[:, :, 0:128], pt[:, :, 0:128],
                                         c_tri[:])
                if pend is not None:
                    emit_av(*pend)
                pend = (kb, pt, qq, n, last)
                if fill is not None and kb in fill:
                    # g=3 region is exp-bound: drop a ready phase-C unit
                    # into the PE instruction stream to cover the stall.
                    fill[kb]()
            emit_av(*pend)
            # Evacuate av banks ASAP (bf16 SBUF copies). Normalization:
            # fast approx reciprocal of the denominator row (f32, direct
            # from PSUM), then one SWDGE DMA that broadcasts it across 64
            # partitions AND casts f32->bf16, then bf16 DVE muls (2x mode).
            avc = nrm.tile([128, 2, 512], BF16, tag="avc")
            dens = nrm.tile([1, 2, 512], F32, tag="dens")
            rec = nrm.tile([1, 2, 512], F32, tag="rec")
            recb = nrm.tile([1, 2, 512], BF16, tag="recb")
            rbs = nrm.tile([64, 2, 512], BF16, tag="rbs")
            tmp = nrm.tile([64, 512], BF16, tag="tmp")
            # custom-DVE ops read SBUF; stage the PSUM denominator row first,
            # split across ACT/DVE so the av banks free up ~0.5us sooner and
            # the reciprocal chain starts earlier.
            nc.scalar.activation(dens[0:1, 0, :], av[64:65, 0, :], AF.Copy)
            nc.vector.tensor_copy(dens[0:1, 1, :], av[64:65, 1, :])
            nc.scalar.activation(avc[0:65, 0, :], av[0:65, 0, :], AF.Copy)
            nc.vector.tensor_copy(avc[0:65, 1, :], av[0:65, 1, :])
            nc.vector.reciprocal_approx_fast(rec[0:1, :, :], dens[0:1, :, :])
            with nc.allow_low_precision(reason="1/denom in bf16: 0.4% uniform row scale, well within tolerance"):
                nc.vector.tensor_copy(recb[0:1, :, :], rec[0:1, :, :])
            nc.gpsimd.partition_broadcast(rbs[0:64, :, :], recb[0:1, :, :])
            nc.vector.tensor_mul(attn[0:64, g, qlo:qlo + 512],
                                 avc[0:64, 0, :], rbs[0:64, 0, :])
            nc.vector.tensor_mul(tmp[0:64, :], avc[0:64, 1, :],
                                 rbs[0:64, 1, :])
            nc.sync.dma_start(attn[64:128, g, qlo:qlo + 512], tmp[0:64, :])

        def phase_c_unit(tp2):
            for _ in range(1):
                o_sb = osb.tile([128, 2, D], BF16, tag="osb")
                for s in range(2):
                    tt = tp2 * 2 + s
                    for h5 in range(2):
                        psO = ps_mix.tile([128, 512], F32, tag="mix")
                        for g in range(4):
                            nc.tensor.matmul(
                                psO[:], attn[:, g, tt * 128:(tt + 1) * 128],
                                w_o[:, g, h5 * 512:(h5 + 1) * 512],
                                start=(g == 0), stop=(g == 3))
                        dst = o_sb[:, s, h5 * 512:(h5 + 1) * 512]
                        # split PSUM evacuation between ACT and DVE so
                        # neither engine becomes the bottleneck
                        if s == 0:
                            nc.scalar.activation(dst, psO[:], AF.Copy)
                        else:
                            nc.vector.tensor_copy(dst, psO[:])
                row = tp2 * 256
                nc.sync.dma_start(
                    out[row:row + 256, :].rearrange("(s p) f -> p s f", p=128),
                    o_sb[:])

        # Emission order: A(g+1) interleaves with B(g) so the PE gap-fills;
        # C(qc) right after B(3, qc) — all four groups' attn cols are ready.
        for tb in range(4):
            v_quarter(tb)
        qk = {0: phase_a(0)}
        for g in range(4):
            # g=3 runs qc=0 first (smallest chunk: nothing can fill the PE
            # while it waits on B(3,0)), then descending so C(qc) work is
            # always available to gap-fill the remaining chunks. C(0) is
            # held back to the very end: its deps are ready long before,
            # so it fills the PE while the last chunk's normalize tail
            # (recip -> broadcast -> mul) finishes.
            if g < 3:
                for qc in range(4):
                    phase_b_chunk(g, *qk[g], qc)
                qk[g + 1] = phase_a(g + 1)
        # g=3: B chunks are exp-bound on ACT (no A work left to gap-fill
        # the PE), so phase-C tp2-units are interleaved into the kb loops
        # as soon as their attn columns are ready. C unit for chunk qc is
        # tp2 in {2qc, 2qc+1}; C(qc) needs B(3,qc)+normalize done.
        cu = phase_c_unit
        phase_b_chunk(3, *qk[3], 0)
        phase_b_chunk(3, *qk[3], 3, fill={8: lambda: cu(0), 12: lambda: cu(1)})
        phase_b_chunk(3, *qk[3], 2, fill={6: lambda: cu(6), 9: lambda: cu(7)})
        phase_b_chunk(3, *qk[3], 1, fill={4: lambda: cu(4)})
        # tail: C2's second unit (ready) covers the last normalize chain,
        # then C(1)'s units close out.
        cu(5)
        cu(2)
        cu(3)

    nc.compile()
    return nc


def _host_prep(x, W_qkv, W_o, token_positions):
    import ml_dtypes
    bf = ml_dtypes.bfloat16
    x = np.asarray(x, np.float32)
    W_qkv = np.asarray(W_qkv, np.float32)
    W_o = np.asarray(W_o, np.float32)
    pos = np.asarray(token_positions, np.float64)
    i = np.arange(32)
    inv = 1.0 / (ROPE_THETA ** (2 * i / DK))
    ang = pos[None, :] * inv[:, None]
    CC = np.tile(np.cos(ang), (4, 1)).astype(bf)
    sn = np.sin(ang)
    SS = np.concatenate([sn, -sn, sn, -sn], 0).astype(bf)
    tri01 = np.where(np.arange(128)[:, None] <= np.arange(128)[None, :],
                     1.0, 0.0).astype(bf)
    tri2 = np.repeat(tri01[:, None, :], 2, axis=1)
    xTb = [np.ascontiguousarray(x[b].T).astype(bf) for b in range(B)]
    in_maps = []
    for c in range(NCORES):
        b, hg = c // 2, c % 2
        qcols, vcols = [], []
        for h in range(hg * 8, hg * 8 + 8):
            for half in range(2):
                qcols.extend(h * DK + 2 * ii + half for ii in range(32))
            vcols.extend(h * DK + d for d in range(DK))
        qcols = np.array(qcols)
        vcols = np.array(vcols)
        in_maps.append({
            "xT": xTb[b],
            "wq": np.ascontiguousarray(W_qkv[:, 0 * D + qcols]).astype(bf),
            "wk": np.ascontiguousarray(W_qkv[:, 1 * D + qcols] / 8.0).astype(bf),
            "wv": np.ascontiguousarray(W_qkv[:, 2 * D + vcols]).astype(bf),
            "wo": np.ascontiguousarray(W_o[vcols, :]).astype(bf),
            "cc": CC, "ss": SS, "tri": tri2,
        })
    return in_maps


def kernel(x, W_qkv, W_o, token_positions, _trace=False):
    in_maps = _host_prep(x, W_qkv, W_o, token_positions)
    if "nc" not in _BUILT:
        _BUILT["nc"] = _build_nc()
    res = run_bass_kernel_spmd(_BUILT["nc"], in_maps,
                               core_ids=list(range(NCORES)), trace=_trace)
    _BUILT["last_result"] = res
    total = np.zeros((B, T, D), np.float32)
    for c in range(NCORES):
        total[c // 2] += np.asarray(res.results[c]["out"], np.float32)
    return total



# revision 28
# speedup vs baseline: 1.3178x; 1.0079x over previous
"""Causal multi-head self-attention with RoPE on 8 Trainium2 NeuronCores.

Sharding: core c handles batch c//2 and heads 8*(c%2) .. 8*(c%2)+8 (half the
heads of one batch). Host sums the two half-head partial outputs per batch
(kernel emits bf16 partials; the sum is f32 on host).

Per-core program (1 batch, 8 heads as 4 groups of 2), bf16 matmuls with fp32
PSUM accumulation:
  V:    V[tok, h*64+d] = x @ Wv directly (no PE transposes) -> vsb bf16,
        with a ones column per head (65th) for softmax denominators.
        Input DMA interleaves (w_v[dt], xT[dt, first 512 toks]) pairs so the
        first V chain streams behind the DMA; ~24 dummy PE matmuls warm the
        HAM clock gate during the wait.
  A(g): Q^T/K^T [128 = 2 heads x (32 even | 32 odd), T] + RoPE
        (cos/sin muls on DVE, cross-term swap via SBUF DMAs).
  B(g): per q-chunk qc (512 q): causal S^T blocks [k=128, q<=512] per head
        (PE row-groups 0-63 / 64-127 co-run), causal mask via trib mul on
        the diagonal staircase, ONE exp per kb on ACT -> pt bf16.
        AV per head: av [128,2,512] PSUM = [values ; denominator row 64]
        (ones column). Normalize: denominator row staged to SBUF (ACT+DVE
        split) -> reciprocal_approx_fast (custom DVE, ~1.2us vs 7.9us for
        InstReciprocal) -> bf16 cast -> partition_broadcast (Pool) -> bf16
        DVE muls (2x mode); odd head lands at partitions 0-63 and is
        DMA-shifted to attn rows 64-127.
  C:    out[tok, :] accumulates attn_g^T @ Wo_g (K=128) over the 4 groups,
        PSUM evacuated on alternating ACT/DVE, bf16 out.
Schedule: A(g+1) emitted after B(g) so its matmuls gap-fill the exp-bound
B stream. g=3 has no A work left, so phase-C tp2-units are interleaved
directly into the B(3, qc) kb loops (qc order 0,3,2,1) as their attn
columns become ready, and the last units cover the final normalize tail.
"""
import numpy as np
from contextlib import ExitStack

import concourse.bass as bass
import concourse.tile as tile
from concourse import bacc, mybir
from concourse.bass_utils import run_bass_kernel_spmd

F32 = mybir.dt.float32
BF16 = mybir.dt.bfloat16
AF = mybir.ActivationFunctionType

D, H, DK, T, B = 1024, 16, 64, 2048, 4
NCORES = 8
ROPE_THETA = 10000.0
_BUILT = {}


def _build_nc():
    nc = bacc.Bacc("TRN2", target_bir_lowering=False, debug=False,
                   num_devices=NCORES)
    xT = nc.dram_tensor("xT", [D, T], BF16, kind="ExternalInput").ap()
    wq = nc.dram_tensor("wq", [D, 512], BF16, kind="ExternalInput").ap()
    wk = nc.dram_tensor("wk", [D, 512], BF16, kind="ExternalInput").ap()
    wv = nc.dram_tensor("wv", [D, 512], BF16, kind="ExternalInput").ap()
    wo = nc.dram_tensor("wo", [512, D], BF16, kind="ExternalInput").ap()
    cc = nc.dram_tensor("cc", [128, T], BF16, kind="ExternalInput").ap()
    ss = nc.dram_tensor("ss", [128, T], BF16, kind="ExternalInput").ap()
    tri = nc.dram_tensor("tri", [128, 2, 128], BF16, kind="ExternalInput").ap()
    out = nc.dram_tensor("out", [T, D], BF16, kind="ExternalOutput").ap()

    x3 = xT.rearrange("(dt p) n -> p dt n", p=128)    # [128, 8, T]
    wq3 = wq.rearrange("(dt p) m -> p dt m", p=128)   # [128, 8, 512]
    wk3 = wk.rearrange("(dt p) m -> p dt m", p=128)
    wv3 = wv.rearrange("(dt p) m -> p dt m", p=128)
    wo3 = wo.rearrange("(g p) m -> p g m", p=128)     # [128, 4, 1024]

    with tile.TileContext(nc) as tc, ExitStack() as ctx:
        consts = ctx.enter_context(tc.tile_pool(name="consts", bufs=1))
        wpool = ctx.enter_context(tc.tile_pool(name="wpool", bufs=1))
        xin = ctx.enter_context(tc.tile_pool(name="xin", bufs=1))
        vtp = ctx.enter_context(tc.tile_pool(name="vtp", bufs=1))
        qkp = ctx.enter_context(tc.tile_pool(name="qkp", bufs=2))
        rope = ctx.enter_context(tc.tile_pool(name="rope", bufs=1))
        atp = ctx.enter_context(tc.tile_pool(name="atp", bufs=1))
        ptp = ctx.enter_context(tc.tile_pool(name="ptp", bufs=4))
        nrm = ctx.enter_context(tc.tile_pool(name="nrm", bufs=3))
        osb = ctx.enter_context(tc.tile_pool(name="osb", bufs=2))
        ps_mix = ctx.enter_context(tc.tile_pool(name="ps_mix", bufs=2, space="PSUM"))
        ps_s = ctx.enter_context(tc.tile_pool(name="ps_s", bufs=2, space="PSUM"))
        ps_av = ctx.enter_context(tc.tile_pool(name="ps_av", bufs=1, space="PSUM"))

        c_cc = consts.tile([128, T], BF16)
        c_ss = consts.tile([128, T], BF16)
        c_tri = consts.tile([128, 2, 128], BF16)
        w_q = wpool.tile([128, 8, 512], BF16)
        w_k = wpool.tile([128, 8, 512], BF16)
        w_v = wpool.tile([128, 8, 512], BF16)
        w_o = wpool.tile([128, 4, D], BF16)
        xt = xin.tile([128, 8, T], BF16)
        vsb = vtp.tile([128, 16, 8, 65], BF16)
        attn = atp.tile([128, 4, T], BF16)

        # V needs w_v + xt first; interleave per-dt (w_v[dt], xt[dt, tb0])
        # pairs so the first V matmul chain can start streaming as soon as
        # the first pairs land, then the rest of xt in 512-col quarters.
        for dt_i in range(8):
            nc.sync.dma_start(w_v[:, dt_i, :], wv3[:, dt_i, :])
            nc.sync.dma_start(xt[:, dt_i, 0:512], x3[:, dt_i, 0:512])
        for tb in range(1, 4):
            c0 = tb * 512
            for dt_i in range(8):
                nc.sync.dma_start(xt[:, dt_i, c0:c0 + 512],
                                  x3[:, dt_i, c0:c0 + 512])
        nc.sync.dma_start(w_q[:], wq3)
        nc.sync.dma_start(c_cc[:], cc)
        nc.sync.dma_start(c_ss[:], ss)
        nc.sync.dma_start(w_k[:], wk3)
        nc.sync.dma_start(c_tri[:], tri)
        nc.sync.dma_start(w_o[:], wo3)

        # ---------------- V: V[tok, h*64+d] = x @ Wv, written pre-transposed
        nc.gpsimd.memset(vsb[:, :, :, 64:65], 1.0)

        # Warm the PE clock (HAM) during the initial input-DMA wait: dummy
        # matmuls on the memset ones column need no DMA. Sized to end just
        # before the first real matmul's inputs land (~10us at ~270ns each)
        # so the HW clock gate is open (and stays open: trailing idle < the
        # 3.4us window).
        ones_col = vsb[:, 0, 0, 64:65]
        ones_row = vsb[:, :, :, 64:65].rearrange("p a b c -> p (a b c)")
        dav = ps_av.tile([128, 2, 512], F32, tag="av")
        for _ in range(24):
            nc.tensor.matmul(dav[0:1, 0, 0:128], ones_col, ones_row,
                             start=True, stop=True, skip_group_check=True)

        def v_quarter(tb):
            for tt in range(4 * tb, 4 * tb + 4):
                psV = ps_mix.tile([128, 512], F32, tag="mix")
                for dt_i in range(8):
                    nc.tensor.matmul(psV[:],
                                     xt[:, dt_i, tt * 128:(tt + 1) * 128],
                                     w_v[:, dt_i, :],
                                     start=(dt_i == 0), stop=(dt_i == 7))
                src = psV[:].rearrange("p (h d) -> p h d", h=8)
                nc.scalar.activation(vsb[:, tt, :, 0:64], src, AF.Copy)

        def phase_a(g, interleave=None):
            qt = qkp.tile([128, T], BF16, tag="qt")
            kt = qkp.tile([128, T], BF16, tag="kt")
            bq = rope.tile([128, T], BF16, tag="bq")
            bk = rope.tile([128, T], BF16, tag="bk")
            bsq = rope.tile([128, T], BF16, tag="bsq")
            bsk = rope.tile([128, T], BF16, tag="bsk")
            bwq = rope.tile([128, T], BF16, tag="bwq")
            bwk = rope.tile([128, T], BF16, tag="bwk")
            for tb in range(4):
                if interleave is not None:
                    interleave(tb)
                lt = tb * 512
                for wsb, bdst, bsdst, stg in ((w_q, bq, bsq, "sq"),
                                              (w_k, bk, bsk, "sk")):
                    psA = ps_mix.tile([128, 512], F32, tag="mix")
                    for dt_i in range(8):
                        nc.tensor.matmul(
                            psA[:], wsb[:, dt_i, g * 128:(g + 1) * 128],
                            xt[:, dt_i, lt:lt + 512],
                            start=(dt_i == 0), stop=(dt_i == 7))
                    # bf16 staging copy on ACT frees the PSUM slot quickly
                    # and lets both RoPE muls run in the DVE 4x mode.
                    stga = rope.tile([128, 512], BF16, tag=stg)
                    nc.vector.tensor_copy(stga[:], psA[:])
                    nc.vector.tensor_mul(bdst[:, lt:lt + 512], stga[:],
                                         c_cc[:, lt:lt + 512])
                    nc.vector.tensor_mul(bsdst[:, lt:lt + 512], stga[:],
                                         c_ss[:, lt:lt + 512])
                if tb in (1, 3):
                    hlo = (tb - 1) * 512
                    for bt, bw, bb, dest in ((bsq, bwq, bq, qt),
                                             (bsk, bwk, bk, kt)):
                        for hh in range(2):
                            r0 = hh * 64
                            nc.sync.dma_start(
                                bw[r0 + 32:r0 + 64, hlo:hlo + 1024],
                                bt[r0:r0 + 32, hlo:hlo + 1024])
                            nc.sync.dma_start(
                                bw[r0:r0 + 32, hlo:hlo + 1024],
                                bt[r0 + 32:r0 + 64, hlo:hlo + 1024])
                        nc.vector.tensor_add(dest[:, hlo:hlo + 1024],
                                             bb[:, hlo:hlo + 1024],
                                             bw[:, hlo:hlo + 1024])
            return qt, kt

        def phase_b_chunk(g, qt, kt, qc, fill=None):
            qlo = qc * 512
            av = ps_av.tile([128, 2, 512], F32, tag="av")
            nkb = 4 * qc + 4

            def emit_av(kb, pt, qq, n, last):
                for h in range(2):
                    nc.tensor.matmul(av[0:65, h, qq:512],
                                     vsb[:, kb, 2 * g + h, :],
                                     pt[:, h, 0:n],
                                     start=(kb == 0), stop=last,
                                     skip_group_check=True)

            # Software-pipelined by one stage: av(kb) is emitted AFTER
            # scores(kb+1) so the in-order PE queue never stalls on exp(kb).
            pend = None
            for kb in range(nkb):
                k0 = kb * 128
                q0 = max(qlo, k0)
                n = qlo + 512 - q0
                qq = q0 - qlo
                diag = (q0 == k0)
                last = (kb == nkb - 1)
                sps = ps_s.tile([128, 2, 512], F32, tag="s")
                for h in range(2):
                    nc.tensor.matmul(
                        sps[:, h, 0:n],
                        kt[h * 64:(h + 1) * 64, k0:k0 + 128],
                        qt[h * 64:(h + 1) * 64, q0:qlo + 512],
                        start=True, stop=True,
                        skip_group_check=True)
                pt = ptp.tile([128, 2, 512], BF16, tag="pt")
                nc.scalar.activation(pt[:, :, 0:n], sps[:, :, 0:n], AF.Exp)
                if diag:
                    # causal mask: zero the upper triangle of the 128-wide
                    # diagonal block (all-bf16 SBUF mul -> DVE 4x mode)
                    nc.vector.tensor_mul(pt[:, :, 0:128], pt[:, :, 0:128],
                                         c_tri[:])
                if pend is not None:
                    emit_av(*pend)
                pend = (kb, pt, qq, n, last)
                if fill is not None and kb in fill:
                    # g=3 region is exp-bound: drop a ready phase-C unit
                    # into the PE instruction stream to cover the stall.
                    fill[kb]()
            emit_av(*pend)
            # Evacuate av banks ASAP (bf16 SBUF copies). Normalization:
            # fast approx reciprocal of the denominator row (f32, direct
            # from PSUM), then one SWDGE DMA that broadcasts it across 64
            # partitions AND casts f32->bf16, then bf16 DVE muls (2x mode).
            avc = nrm.tile([128, 2, 512], BF16, tag="avc")
            dens = nrm.tile([1, 2, 512], F32, tag="dens")
            rec = nrm.tile([1, 2, 512], F32, tag="rec")
            recb = nrm.tile([1, 2, 512], BF16, tag="recb")
            rbs = nrm.tile([64, 2, 512], BF16, tag="rbs")
            tmp = nrm.tile([64, 512], BF16, tag="tmp")
            # custom-DVE ops read SBUF; stage the PSUM denominator row first,
            # split across ACT/DVE so the av banks free up ~0.5us sooner and
            # the reciprocal chain starts earlier.
            nc.scalar.activation(dens[0:1, 0, :], av[64:65, 0, :], AF.Copy)
            nc.vector.tensor_copy(dens[0:1, 1, :], av[64:65, 1, :])
            nc.scalar.activation(avc[0:65, 0, :], av[0:65, 0, :], AF.Copy)
            nc.vector.tensor_copy(avc[0:65, 1, :], av[0:65, 1, :])
            nc.vector.reciprocal_approx_fast(rec[0:1, :, :], dens[0:1, :, :])
            with nc.allow_low_precision(reason="1/denom in bf16: 0.4% uniform row scale, well within tolerance"):
                nc.vector.tensor_copy(recb[0:1, :, :], rec[0:1, :, :])
            nc.gpsimd.partition_broadcast(rbs[0:64, :, :], recb[0:1, :, :])
            nc.vector.tensor_mul(attn[0:64, g, qlo:qlo + 512],
                                 avc[0:64, 0, :], rbs[0:64, 0, :])
            nc.vector.tensor_mul(tmp[0:64, :], avc[0:64, 1, :],
                                 rbs[0:64, 1, :])
            nc.sync.dma_start(attn[64:128, g, qlo:qlo + 512], tmp[0:64, :])

        def phase_c_unit(tp2):
            for _ in range(1):
                o_sb = osb.tile([128, 2, D], BF16, tag="osb")
                for s in range(2):
                    tt = tp2 * 2 + s
                    for h5 in range(2):
                        psO = ps_mix.tile([128, 512], F32, tag="mix")
                        for g in range(4):
                            nc.tensor.matmul(
                                psO[:], attn[:, g, tt * 128:(tt + 1) * 128],
                                w_o[:, g, h5 * 512:(h5 + 1) * 512],
                                start=(g == 0), stop=(g == 3))
                        dst = o_sb[:, s, h5 * 512:(h5 + 1) * 512]
                        # split PSUM evacuation between ACT and DVE so
                        # neither engine becomes the bottleneck
                        if s == 0:
                            nc.scalar.activation(dst, psO[:], AF.Copy)
                        else:
                            nc.vector.tensor_copy(dst, psO[:])
                row = tp2 * 256
                nc.sync.dma_start(
                    out[row:row + 256, :].rearrange("(s p) f -> p s f", p=128),
                    o_sb[:])

        # Emission order: A(g+1) interleaves with B(g) so the PE gap-fills;
        # C(qc) right after B(3, qc) — all four groups' attn cols are ready.
        for tb in range(4):
            v_quarter(tb)
        qk = {0: phase_a(0)}
        for g in range(4):
            # g=3 runs qc=0 first (smallest chunk: nothing can fill the PE
            # while it waits on B(3,0)), then descending so C(qc) work is
            # always available to gap-fill the remaining chunks. C(0) is
            # held back to the very end: its deps are ready long before,
            # so it fills the PE while the last chunk's normalize tail
            # (recip -> broadcast -> mul) finishes.
            if g < 3:
                for qc in range(4):
                    phase_b_chunk(g, *qk[g], qc)
                qk[g + 1] = phase_a(g + 1)
        # g=3: B chunks are exp-bound on ACT (no A work left to gap-fill
        # the PE), so phase-C tp2-units are interleaved into the kb loops
        # as soon as their attn columns are ready. C unit for chunk qc is
        # tp2 in {2qc, 2qc+1}; C(qc) needs B(3,qc)+normalize done.
        cu = phase_c_unit
        phase_b_chunk(3, *qk[3], 0)
        phase_b_chunk(3, *qk[3], 3, fill={8: lambda: cu(0), 12: lambda: cu(1)})
        phase_b_chunk(3, *qk[3], 2, fill={6: lambda: cu(6), 9: lambda: cu(7)})
        phase_b_chunk(3, *qk[3], 1, fill={4: lambda: cu(4)})
        # tail: C2's second unit (ready) covers the last normalize chain,
        # then C(1)'s units close out.
        cu(5)
        cu(2)
        cu(3)

    nc.compile()
    return nc


def _host_prep(x, W_qkv, W_o, token_positions):
    import ml_dtypes
    bf = ml_dtypes.bfloat16
    x = np.asarray(x, np.float32)
    W_qkv = np.asarray(W_qkv, np.float32)
    W_o = np.asarray(W_o, np.float32)
    pos = np.asarray(token_positions, np.float64)
    i = np.arange(32)
    inv = 1.0 / (ROPE_THETA ** (2 * i / DK))
    ang = pos[None, :] * inv[:, None]
    CC = np.tile(np.cos(ang), (4, 1)).astype(bf)
    sn = np.sin(ang)
    SS = np.concatenate([sn, -sn, sn, -sn], 0).astype(bf)
    tri01 = np.where(np.arange(128)[:, None] <= np.arange(128)[None, :],
                     1.0, 0.0).astype(bf)
    tri2 = np.repeat(tri01[:, None, :], 2, axis=1)
    xTb = [np.ascontiguousarray(x[b].T).astype(bf) for b in range(B)]
    in_maps = []
    for c in range(NCORES):
        b, hg = c // 2, c % 2
        qcols, vcols = [], []
        for h in range(hg * 8, hg * 8 + 8):
            for half in range(2):
                qcols.extend(h * DK + 2 * ii + half for ii in range(32))
            vcols.extend(h * DK + d for d in range(DK))
        qcols = np.array(qcols)
        vcols = np.array(vcols)
        in_maps.append({
            "xT": xTb[b],
            "wq": np.ascontiguousarray(W_qkv[:, 0 * D + qcols]).astype(bf),
            "wk": np.ascontiguousarray(W_qkv[:, 1 * D + qcols] / 8.0).astype(bf),
            "wv": np.ascontiguousarray(W_qkv[:, 2 * D + vcols]).astype(bf),
            "wo": np.ascontiguousarray(W_o[vcols, :]).astype(bf),
            "cc": CC, "ss": SS, "tri": tri2,
        })
    return in_maps


def kernel(x, W_qkv, W_o, token_positions, _trace=False):
    in_maps = _host_prep(x, W_qkv, W_o, token_positions)
    if "nc" not in _BUILT:
        _BUILT["nc"] = _build_nc()
    res = run_bass_kernel_spmd(_BUILT["nc"], in_maps,
                               core_ids=list(range(NCORES)), trace=_trace)
    _BUILT["last_result"] = res
    total = np.zeros((B, T, D), np.float32)
    for c in range(NCORES):
        total[c // 2] += np.asarray(res.results[c]["out"], np.float32)
    return total

